# revision 8
# baseline (speedup 1.0000x reference)
"""Trainium2 Bass kernel for batched multi-slice attention.

Reference computation (per (b, l) slice, C=S=256, D=512):
    q = queries @ Wq.T + bq
    k = keys @ Wkv.T + bkv
    v = values @ Wkv.T + bkv
    attn = softmax(q @ k.T / sqrt(D))
    out = (attn @ v) @ Wo.T + bo

Sharding: B*L = 128 independent slices, 16 per core across 8 NeuronCores
(data parallel); the D x D projection weights are replicated.

All matmuls run in fp32r (fp32 with 11-bit mantissa, full PE rate at
free-dim >= 256). Everything is computed in "transposed" activation
layout so no on-chip transposes are needed:
  qT[d',c], kT[d',s]  (partition = d' chunk)   from host-transposed inputs
  v[s,d']             (partition = s chunk)
  scoresT[s,c] = kT.T @ qT ; expT = exp(scale*scoresT)  (no max needed:
       scaled scores are ~N(0,1), exp cannot overflow fp32)
  sums[c,1] = expT.T @ ones  (matmul-with-ones puts softmax denominator
       directly into partition-per-c layout)
  attnT[d',c] = v.T @ expT   (unnormalized)
  out[c,do] = attnT.T @ WoT, then * (1/sums[c]) per-partition + bo
"""
import numpy as np
from contextlib import ExitStack

N_CORES = 8
B, L, C, S, D = 2, 64, 256, 256, 512
SLICES = B * L              # 128
PER_CORE = SLICES // N_CORES  # 16
PAIRS = PER_CORE // 2       # 8
SCALE = 1.0 / np.sqrt(np.float32(D))

_COMPILED = None


def _build():
    import concourse.mybir as mybir
    import concourse.tile as tile
    from concourse import bacc

    F32R = mybir.dt.float32r
    F32 = mybir.dt.float32
    EXP = mybir.ActivationFunctionType.Exp

    nc = bacc.Bacc("TRN2", target_bir_lowering=False, debug=False)

    # DRAM I/O (per-core shard)
    xq = nc.dram_tensor("xq", [PER_CORE, D, C], F32R, kind="ExternalInput").ap()
    xk = nc.dram_tensor("xk", [PER_CORE, D, C], F32R, kind="ExternalInput").ap()
    xv = nc.dram_tensor("xv", [PER_CORE, D, C], F32R, kind="ExternalInput").ap()
    wqT = nc.dram_tensor("wqT", [D, D], F32R, kind="ExternalInput").ap()
    wkvT = nc.dram_tensor("wkvT", [D, D], F32R, kind="ExternalInput").ap()
    woT = nc.dram_tensor("woT", [D, D], F32R, kind="ExternalInput").ap()
    bq_d = nc.dram_tensor("bq_c", [4, 128], F32, kind="ExternalInput").ap()
    bkv_d = nc.dram_tensor("bkv_c", [4, 128], F32, kind="ExternalInput").ap()
    bkv_f = nc.dram_tensor("bkv_f", [1, D], F32, kind="ExternalInput").ap()
    bo_f = nc.dram_tensor("bo_f", [1, D], F32, kind="ExternalInput").ap()
    ones_c = nc.dram_tensor("ones_c", [128, 1], F32, kind="ExternalInput").ap()
    ones_r = nc.dram_tensor("ones_r", [1, 128], F32, kind="ExternalInput").ap()
    out = nc.dram_tensor("out", [PER_CORE, C, D], F32, kind="ExternalOutput").ap()

    with tile.TileContext(nc) as tc, ExitStack() as ctx:
        const = ctx.enter_context(tc.tile_pool(name="const", bufs=1))
        inp = ctx.enter_context(tc.tile_pool(name="inp", bufs=2))
        work = ctx.enter_context(tc.tile_pool(name="work", bufs=2))
        ps512 = ctx.enter_context(tc.tile_pool(name="ps512", bufs=4, space="PSUM"))
        ps256 = ctx.enter_context(tc.tile_pool(name="ps256", bufs=3, space="PSUM"))
        pssum = ctx.enter_context(tc.tile_pool(name="pssum", bufs=1, space="PSUM"))

        # ---- constants ----
        wq_sb = const.tile([128, 4 * D], F32R, tag="wq")
        wkv_sb = const.tile([128, 4 * D], F32R, tag="wkv")
        wo_sb = const.tile([128, 4 * D], F32R, tag="wo")
        for w_sb, w_dram in ((wq_sb, wqT), (wkv_sb, wkvT), (wo_sb, woT)):
            nc.sync.dma_start(
                w_sb[:].rearrange("p (j n) -> p j n", j=4),
                w_dram.rearrange("(j p) n -> p j n", p=128),
            )
        bq_sb = const.tile([128, 4], F32, tag="bq")
        bkv_sb = const.tile([128, 4], F32, tag="bkv")
        nc.sync.dma_start(bq_sb[:], bq_d.rearrange("j p -> p j"))
        nc.sync.dma_start(bkv_sb[:], bkv_d.rearrange("j p -> p j"))
        bkv_fl = const.tile([1, D], F32, tag="bkvf")
        bo_fl = const.tile([1, D], F32, tag="bof")
        nc.sync.dma_start(bkv_fl[:], bkv_f)
        nc.sync.dma_start(bo_fl[:], bo_f)
        ones_col = const.tile([128, 1], F32, tag="onesc")
        ones_row = const.tile([1, 128], F32, tag="onesr")
        nc.sync.dma_start(ones_col[:], ones_c)
        nc.sync.dma_start(ones_row[:], ones_r)

        # broadcast bkv/bo across partitions via ones outer product (fp32)
        bkv_bc = const.tile([128, D], F32, tag="bkvbc")
        bo_bc = const.tile([128, D], F32, tag="bobc")
        for bc, fl in ((bkv_bc, bkv_fl), (bo_bc, bo_fl)):
            p = ps512.tile([128, D], F32, tag="mm512")
            nc.tensor.matmul(p[:], ones_row[:], fl[:], start=True, stop=True)
            nc.vector.tensor_copy(bc[:], p[:])

        # ---- main loop: 8 pairs of slices ----
        for pr in range(PAIRS):
            s0 = 2 * pr
            # load pair inputs: sbuf[p, j*512 + sl*256 + c] = src[s0+sl, j*128+p, c]
            xq_sb = inp.tile([128, 2048], F32R, tag="xq")
            xk_sb = inp.tile([128, 2048], F32R, tag="xk")
            xv_sb = inp.tile([128, 2048], F32R, tag="xv")
            for sb, dram in ((xq_sb, xq), (xk_sb, xk), (xv_sb, xv)):
                sb_pjc = sb[:].rearrange("p (j c2) -> p j c2", c2=2 * C)
                for sl in range(2):
                    nc.sync.dma_start(
                        sb_pjc[:, :, sl * C : (sl + 1) * C],
                        dram[s0 + sl].rearrange("(j p) c -> p j c", p=128),
                    )

            # q/k projections, pair-batched (free = sl*256+c = 512):
            # qT[d'=m*128+p, sl, c] = sum_d WqT[d, d'] * x[d, sl, c]
            qT = work.tile([128, 2048], F32R, tag="qT")
            kT = work.tile([128, 2048], F32R, tag="kT")
            for dst, w_sb, x_sb, b_sb in (
                (qT, wq_sb, xq_sb, bq_sb),
                (kT, wkv_sb, xk_sb, bkv_sb),
            ):
                for m in range(4):
                    p = ps512.tile([128, 512], F32, tag="mm512")
                    for j in range(4):
                        nc.tensor.matmul(
                            p[:],
                            w_sb[:, j * 512 + m * 128 : j * 512 + m * 128 + 128],
                            x_sb[:, j * 512 : (j + 1) * 512],
                            start=(j == 0),
                            stop=(j == 3),
                        )
                    nc.vector.tensor_scalar_add(
                        dst[:, m * 512 : (m + 1) * 512], p[:], b_sb[:, m : m + 1]
                    )

            # v projection, natural layout: v[s = sc*128+p, d'] per slice sl
            v_sb = work.tile([128, 2048], F32R, tag="v")
            for sl in range(2):
                for sc in range(2):
                    p = ps512.tile([128, 512], F32, tag="mm512")
                    for j in range(4):
                        nc.tensor.matmul(
                            p[:],
                            xv_sb[:, j * 512 + sl * 256 + sc * 128 :
                                  j * 512 + sl * 256 + sc * 128 + 128],
                            wkv_sb[:, j * 512 : (j + 1) * 512],
                            start=(j == 0),
                            stop=(j == 3),
                        )
                    nc.vector.tensor_add(
                        v_sb[:, (sl * 2 + sc) * 512 : (sl * 2 + sc + 1) * 512],
                        p[:],
                        bkv_bc[:],
                    )

            # scoresT[s, c] per slice; exp -> expT_sb
            expT = work.tile([128, 1024], F32R, tag="expT")
            for sl in range(2):
                for sc in range(2):
                    p = ps256.tile([128, 256], F32, tag="mm256")
                    for j in range(4):
                        base = j * 512 + sl * 256
                        nc.tensor.matmul(
                            p[:],
                            kT[:, base + sc * 128 : base + sc * 128 + 128],
                            qT[:, base : base + 256],
                            start=(j == 0),
                            stop=(j == 3),
                        )
                    nc.scalar.activation(
                        expT[:, sl * 512 + sc * 256 : sl * 512 + sc * 256 + 256],
                        p[:],
                        EXP,
                        scale=float(SCALE),
                    )

            # softmax denominators straight into [c-partition, 1] layout
            recip = work.tile([128, 4], F32, tag="recip")
            for sl in range(2):
                for cc in range(2):
                    p = pssum.tile([128, 1], F32, tag="sums")
                    for sc in range(2):
                        nc.tensor.matmul(
                            p[:],
                            expT[:, sl * 512 + sc * 256 + cc * 128 :
                                 sl * 512 + sc * 256 + cc * 128 + 128
                                 ].bitcast(F32),
                            ones_col[:],
                            start=(sc == 0),
                            stop=(sc == 1),
                        )
                    nc.vector.reciprocal(recip[:, sl * 2 + cc : sl * 2 + cc + 1], p[:])

            # attnT[d' = m*128+p, c] = sum_s v[s, d'] * expT[s, c]  (unnormalized)
            attnT = work.tile([128, 2048], F32R, tag="attnT")
            for sl in range(2):
                for m in range(4):
                    p = ps256.tile([128, 256], F32, tag="mm256")
                    for sc in range(2):
                        nc.tensor.matmul(
                            p[:],
                            v_sb[:, (sl * 2 + sc) * 512 + m * 128 :
                                 (sl * 2 + sc) * 512 + m * 128 + 128],
                            expT[:, sl * 512 + sc * 256 : sl * 512 + sc * 256 + 256],
                            start=(sc == 0),
                            stop=(sc == 1),
                        )
                    nc.vector.tensor_copy(
                        attnT[:, m * 512 + sl * 256 : m * 512 + sl * 256 + 256], p[:]
                    )

            # final: out[c = cc*128+p, do] = sum_d' attnT[d', c] * WoT[d', do]
            o_sb = work.tile([128, 2048], F32, tag="osb")
            for sl in range(2):
                for cc in range(2):
                    p = ps512.tile([128, 512], F32, tag="mm512")
                    for j in range(4):
                        nc.tensor.matmul(
                            p[:],
                            attnT[:, j * 512 + sl * 256 + cc * 128 :
                                  j * 512 + sl * 256 + cc * 128 + 128],
                            wo_sb[:, j * 512 : (j + 1) * 512],
                            start=(j == 0),
                            stop=(j == 3),
                        )
                    o_slice = o_sb[:, (sl * 2 + cc) * 512 : (sl * 2 + cc + 1) * 512]
                    nc.vector.tensor_scalar_mul(
                        o_slice, p[:], recip[:, sl * 2 + cc : sl * 2 + cc + 1]
                    )
                    nc.vector.tensor_add(o_slice, o_slice, bo_bc[:])
                    nc.sync.dma_start(
                        out[s0 + sl, cc * 128 : (cc + 1) * 128, :], o_slice
                    )

    nc.compile()
    return nc


def _get_compiled():
    global _COMPILED
    if _COMPILED is None:
        _COMPILED = _build()
    return _COMPILED


def kernel(queries, keys, values, Wq, bq, Wkv, bkv, Wo, bo):
    from concourse.bass_utils import run_bass_kernel_spmd

    nc = _get_compiled()

    f32 = np.float32
    qT = np.ascontiguousarray(
        np.asarray(queries, dtype=f32).reshape(SLICES, C, D).transpose(0, 2, 1)
    )
    kTx = np.ascontiguousarray(
        np.asarray(keys, dtype=f32).reshape(SLICES, S, D).transpose(0, 2, 1)
    )
    vTx = np.ascontiguousarray(
        np.asarray(values, dtype=f32).reshape(SLICES, S, D).transpose(0, 2, 1)
    )
    shared = {
        "wqT": np.ascontiguousarray(np.asarray(Wq, dtype=f32).T),
        "wkvT": np.ascontiguousarray(np.asarray(Wkv, dtype=f32).T),
        "woT": np.ascontiguousarray(np.asarray(Wo, dtype=f32).T),
        "bq_c": np.ascontiguousarray(np.asarray(bq, dtype=f32).reshape(4, 128)),
        "bkv_c": np.ascontiguousarray(np.asarray(bkv, dtype=f32).reshape(4, 128)),
        "bkv_f": np.ascontiguousarray(np.asarray(bkv, dtype=f32).reshape(1, D)),
        "bo_f": np.ascontiguousarray(np.asarray(bo, dtype=f32).reshape(1, D)),
        "ones_c": np.ones((128, 1), dtype=f32),
        "ones_r": np.ones((1, 128), dtype=f32),
    }
    in_maps = []
    for c in range(N_CORES):
        sl = slice(c * PER_CORE, (c + 1) * PER_CORE)
        in_maps.append(
            {"xq": qT[sl], "xk": kTx[sl], "xv": vTx[sl], **shared}
        )

    res = run_bass_kernel_spmd(nc, in_maps, core_ids=list(range(N_CORES)))
    full = np.concatenate([res.results[c]["out"] for c in range(N_CORES)], axis=0)
    return full.reshape(B, L, C, D).astype(np.float32, copy=False)


# revision 9
# speedup vs baseline: 1.1161x; 1.1161x over previous
"""Trainium2 Bass kernel for batched multi-slice attention.

Reference computation (per (b, l) slice, C=S=256, D=512):
    q = queries @ Wq.T + bq
    k = keys @ Wkv.T + bkv
    v = values @ Wkv.T + bkv
    attn = softmax(q @ k.T / sqrt(D))
    out = (attn @ v) @ Wo.T + bo

Sharding: B*L = 128 independent slices, 16 per core across 8 NeuronCores
(data parallel); weights are replicated.

Fast path (all biases zero — always true for this problem's inputs):
algebraic refactor that folds the projection weights into two
precomputed DxD products (host-side, batch-independent):
    A = Wq.T @ Wkv          ->  scores = x_q @ A @ x_k.T
    Bm = (Wo @ Wkv).T       ->  out    = softmax(scores/sqrt(D)) @ x_v @ Bm
This removes the k and v projections entirely: 402 vs 670 MFLOP/slice.

All matmuls run in fp32r (fp32 with 11-bit mantissa, full PE rate).
Activations stay in "transposed" layout so no on-chip transposes occur:
    tT[g,c] = A.T @ x_qT                  (partition = g chunk)
    scoresT[s,c] = x_kT.T @ tT ; expT = exp(scale*scoresT)  (no max
        subtraction needed: scaled scores are ~N(0,1); fp32 exp is safe)
    sums[c,2] = expT.T @ ones             (softmax denominator directly in
        partition-per-c layout; fp32r needs moving free >= 2)
    uT[d,c] = x_v.T' @ expT               (x_v used in natural layout)
    out[c,do] = uT.T @ Bm, then * (1/sums[c]) per-partition scalar

General path (any nonzero bias): direct implementation with explicit
q/k/v projections and bias adds.
"""
import numpy as np
from contextlib import ExitStack

N_CORES = 8
B, L, C, S, D = 2, 64, 256, 256, 512
SLICES = B * L
PER_CORE = SLICES // N_CORES  # 16
PAIRS = PER_CORE // 2         # 8
SCALE = 1.0 / np.sqrt(np.float32(D))

_COMPILED = {}


def _build_fast():
    import concourse.mybir as mybir
    import concourse.tile as tile
    from concourse import bacc

    F32R = mybir.dt.float32r
    F32 = mybir.dt.float32
    EXP = mybir.ActivationFunctionType.Exp
    COPY = mybir.ActivationFunctionType.Copy

    nc = bacc.Bacc("TRN2", target_bir_lowering=False, debug=False)

    xq = nc.dram_tensor("xq", [PER_CORE, D, C], F32R, kind="ExternalInput").ap()
    xk = nc.dram_tensor("xk", [PER_CORE, D, C], F32R, kind="ExternalInput").ap()
    xv = nc.dram_tensor("xv", [PER_CORE, S, D], F32R, kind="ExternalInput").ap()
    A_d = nc.dram_tensor("A", [D, D], F32R, kind="ExternalInput").ap()
    B_d = nc.dram_tensor("Bm", [D, D], F32R, kind="ExternalInput").ap()
    ones_d = nc.dram_tensor("ones2", [128, 2], F32R, kind="ExternalInput").ap()
    out = nc.dram_tensor("out", [PER_CORE, C, D], F32, kind="ExternalOutput").ap()

    with tile.TileContext(nc) as tc, ExitStack() as ctx:
        const = ctx.enter_context(tc.tile_pool(name="const", bufs=1))
        inp = ctx.enter_context(tc.tile_pool(name="inp", bufs=2))
        work = ctx.enter_context(tc.tile_pool(name="work", bufs=2))
        ps512 = ctx.enter_context(tc.tile_pool(name="ps512", bufs=4, space="PSUM"))
        ps256 = ctx.enter_context(tc.tile_pool(name="ps256", bufs=3, space="PSUM"))
        pssum = ctx.enter_context(tc.tile_pool(name="pssum", bufs=1, space="PSUM"))

        # constants: A first (needed immediately), Bm later, ones tiny
        A_sb = const.tile([128, 4 * D], F32R, tag="A")
        nc.sync.dma_start(
            A_sb[:].rearrange("p (j n) -> p j n", j=4),
            A_d.rearrange("(j p) n -> p j n", p=128),
        )
        ones_sb = const.tile([128, 2], F32R, tag="ones2")
        nc.sync.dma_start(ones_sb[:], ones_d)
        B_sb = const.tile([128, 4 * D], F32R, tag="Bm")
        nc.sync.dma_start(
            B_sb[:].rearrange("p (j n) -> p j n", j=4),
            B_d.rearrange("(j p) n -> p j n", p=128),
        )

        for pr in range(PAIRS):
            s0 = 2 * pr
            # transposed q/k loads: sb[p, j*512 + sl*256 + c] = src[s0+sl, j*128+p, c]
            xq_sb = inp.tile([128, 2048], F32R, tag="xq")
            xk_sb = inp.tile([128, 2048], F32R, tag="xk")
            for sb, dram in ((xq_sb, xq), (xk_sb, xk)):
                sb_pjc = sb[:].rearrange("p (j c2) -> p j c2", c2=2 * C)
                for sl in range(2):
                    nc.sync.dma_start(
                        sb_pjc[:, :, sl * C : (sl + 1) * C],
                        dram[s0 + sl].rearrange("(j p) c -> p j c", p=128),
                    )
            # natural v load: sb[p, sl*1024 + sc*512 + d] = src[s0+sl, sc*128+p, d]
            xv_sb = inp.tile([128, 2048], F32R, tag="xv")
            xv_psd = xv_sb[:].rearrange("p (sl sc d) -> sl p sc d", sl=2, sc=2)
            for sl in range(2):
                nc.sync.dma_start(
                    xv_psd[sl],
                    xv[s0 + sl].rearrange("(sc p) d -> p sc d", p=128),
                )

            # tT[g = m*128+p, (sl,c)] = sum_d A[d, g] * x_q[d, (sl,c)]
            tT = work.tile([128, 2048], F32R, tag="tT")
            for m in range(4):
                p = ps512.tile([128, 512], F32, tag="mm512")
                for j in range(4):
                    nc.tensor.matmul(
                        p[:],
                        A_sb[:, j * 512 + m * 128 : j * 512 + m * 128 + 128],
                        xq_sb[:, j * 512 : (j + 1) * 512],
                        start=(j == 0),
                        stop=(j == 3),
                    )
                nc.vector.tensor_copy(tT[:, m * 512 : (m + 1) * 512], p[:])

            # scoresT[s, c] per slice; exp -> expT
            expT = work.tile([128, 1024], F32R, tag="expT")
            for sl in range(2):
                for sc in range(2):
                    p = ps256.tile([128, 256], F32, tag="mm256")
                    for j in range(4):
                        base = j * 512 + sl * 256
                        nc.tensor.matmul(
                            p[:],
                            xk_sb[:, base + sc * 128 : base + sc * 128 + 128],
                            tT[:, base : base + 256],
                            start=(j == 0),
                            stop=(j == 3),
                        )
                    nc.scalar.activation(
                        expT[:, sl * 512 + sc * 256 : sl * 512 + sc * 256 + 256],
                        p[:],
                        EXP,
                        scale=float(SCALE),
                    )

            # softmax denominators straight into [c-partition, .] layout
            recip = work.tile([128, 4], F32, tag="recip")
            for sl in range(2):
                for cc in range(2):
                    p = pssum.tile([128, 2], F32, tag="sums")
                    for sc in range(2):
                        nc.tensor.matmul(
                            p[:],
                            expT[:, sl * 512 + sc * 256 + cc * 128 :
                                 sl * 512 + sc * 256 + cc * 128 + 128],
                            ones_sb[:],
                            start=(sc == 0),
                            stop=(sc == 1),
                        )
                    nc.vector.reciprocal(
                        recip[:, sl * 2 + cc : sl * 2 + cc + 1], p[:, 0:1]
                    )

            # uT[d = m*128+p, c] = sum_s x_v[s, d] * expT[s, c]  (unnormalized)
            uT = work.tile([128, 2048], F32R, tag="uT")
            for sl in range(2):
                for m in range(4):
                    p = ps256.tile([128, 256], F32, tag="mm256")
                    for sc in range(2):
                        nc.tensor.matmul(
                            p[:],
                            xv_sb[:, sl * 1024 + sc * 512 + m * 128 :
                                  sl * 1024 + sc * 512 + m * 128 + 128],
                            expT[:, sl * 512 + sc * 256 : sl * 512 + sc * 256 + 256],
                            start=(sc == 0),
                            stop=(sc == 1),
                        )
                    # drain on ACT (DVE is the busier engine)
                    nc.scalar.activation(
                        uT[:, m * 512 + sl * 256 : m * 512 + sl * 256 + 256],
                        p[:],
                        COPY,
                    )

            # out[c = cc*128+p, do] = (sum_g uT[g, c] * Bm[g, do]) / sums[c]
            o_sb = work.tile([128, 2048], F32, tag="osb")
            for sl in range(2):
                for cc in range(2):
                    p = ps512.tile([128, 512], F32, tag="mm512")
                    for j in range(4):
                        nc.tensor.matmul(
                            p[:],
                            uT[:, j * 512 + sl * 256 + cc * 128 :
                               j * 512 + sl * 256 + cc * 128 + 128],
                            B_sb[:, j * 512 : (j + 1) * 512],
                            start=(j == 0),
                            stop=(j == 3),
                        )
                    o_slice = o_sb[:, (sl * 2 + cc) * 512 : (sl * 2 + cc + 1) * 512]
                    nc.vector.tensor_scalar_mul(
                        o_slice, p[:], recip[:, sl * 2 + cc : sl * 2 + cc + 1]
                    )
                    nc.sync.dma_start(
                        out[s0 + sl, cc * 128 : (cc + 1) * 128, :], o_slice
                    )

    nc.compile()
    return nc


def _build_general():
    import concourse.mybir as mybir
    import concourse.tile as tile
    from concourse import bacc

    F32R = mybir.dt.float32r
    F32 = mybir.dt.float32
    EXP = mybir.ActivationFunctionType.Exp

    nc = bacc.Bacc("TRN2", target_bir_lowering=False, debug=False)

    xq = nc.dram_tensor("xq", [PER_CORE, D, C], F32R, kind="ExternalInput").ap()
    xk = nc.dram_tensor("xk", [PER_CORE, D, C], F32R, kind="ExternalInput").ap()
    xv = nc.dram_tensor("xv", [PER_CORE, D, C], F32R, kind="ExternalInput").ap()
    wqT = nc.dram_tensor("wqT", [D, D], F32R, kind="ExternalInput").ap()
    wkvT = nc.dram_tensor("wkvT", [D, D], F32R, kind="ExternalInput").ap()
    woT = nc.dram_tensor("woT", [D, D], F32R, kind="ExternalInput").ap()
    bq_d = nc.dram_tensor("bq_c", [4, 128], F32, kind="ExternalInput").ap()
    bkv_d = nc.dram_tensor("bkv_c", [4, 128], F32, kind="ExternalInput").ap()
    bkv_f = nc.dram_tensor("bkv_f", [1, D], F32, kind="ExternalInput").ap()
    bo_f = nc.dram_tensor("bo_f", [1, D], F32, kind="ExternalInput").ap()
    ones_c = nc.dram_tensor("ones_c", [128, 2], F32R, kind="ExternalInput").ap()
    ones_r = nc.dram_tensor("ones_r", [1, 128], F32, kind="ExternalInput").ap()
    out = nc.dram_tensor("out", [PER_CORE, C, D], F32, kind="ExternalOutput").ap()

    with tile.TileContext(nc) as tc, ExitStack() as ctx:
        const = ctx.enter_context(tc.tile_pool(name="const", bufs=1))
        inp = ctx.enter_context(tc.tile_pool(name="inp", bufs=2))
        work = ctx.enter_context(tc.tile_pool(name="work", bufs=2))
        ps512 = ctx.enter_context(tc.tile_pool(name="ps512", bufs=4, space="PSUM"))
        ps256 = ctx.enter_context(tc.tile_pool(name="ps256", bufs=3, space="PSUM"))
        pssum = ctx.enter_context(tc.tile_pool(name="pssum", bufs=1, space="PSUM"))

        wq_sb = const.tile([128, 4 * D], F32R, tag="wq")
        wkv_sb = const.tile([128, 4 * D], F32R, tag="wkv")
        wo_sb = const.tile([128, 4 * D], F32R, tag="wo")
        for w_sb, w_dram in ((wq_sb, wqT), (wkv_sb, wkvT), (wo_sb, woT)):
            nc.sync.dma_start(
                w_sb[:].rearrange("p (j n) -> p j n", j=4),
                w_dram.rearrange("(j p) n -> p j n", p=128),
            )
        bq_sb = const.tile([128, 4], F32, tag="bq")
        bkv_sb = const.tile([128, 4], F32, tag="bkv")
        nc.sync.dma_start(bq_sb[:], bq_d.rearrange("j p -> p j"))
        nc.sync.dma_start(bkv_sb[:], bkv_d.rearrange("j p -> p j"))
        bkv_fl = const.tile([1, D], F32, tag="bkvf")
        bo_fl = const.tile([1, D], F32, tag="bof")
        nc.sync.dma_start(bkv_fl[:], bkv_f)
        nc.sync.dma_start(bo_fl[:], bo_f)
        ones_col = const.tile([128, 2], F32R, tag="onesc")
        ones_row = const.tile([1, 128], F32, tag="onesr")
        nc.sync.dma_start(ones_col[:], ones_c)
        nc.sync.dma_start(ones_row[:], ones_r)

        bkv_bc = const.tile([128, D], F32, tag="bkvbc")
        bo_bc = const.tile([128, D], F32, tag="bobc")
        for bc, fl in ((bkv_bc, bkv_fl), (bo_bc, bo_fl)):
            p = ps512.tile([128, D], F32, tag="mm512")
            nc.tensor.matmul(p[:], ones_row[:], fl[:], start=True, stop=True)
            nc.vector.tensor_copy(bc[:], p[:])

        for pr in range(PAIRS):
            s0 = 2 * pr
            xq_sb = inp.tile([128, 2048], F32R, tag="xq")
            xk_sb = inp.tile([128, 2048], F32R, tag="xk")
            xv_sb = inp.tile([128, 2048], F32R, tag="xv")
            for sb, dram in ((xq_sb, xq), (xk_sb, xk), (xv_sb, xv)):
                sb_pjc = sb[:].rearrange("p (j c2) -> p j c2", c2=2 * C)
                for sl in range(2):
                    nc.sync.dma_start(
                        sb_pjc[:, :, sl * C : (sl + 1) * C],
                        dram[s0 + sl].rearrange("(j p) c -> p j c", p=128),
                    )

            qT = work.tile([128, 2048], F32R, tag="qT")
            kT = work.tile([128, 2048], F32R, tag="kT")
            for dst, w_sb, x_sb, b_sb in (
                (qT, wq_sb, xq_sb, bq_sb),
                (kT, wkv_sb, xk_sb, bkv_sb),
            ):
                for m in range(4):
                    p = ps512.tile([128, 512], F32, tag="mm512")
                    for j in range(4):
                        nc.tensor.matmul(
                            p[:],
                            w_sb[:, j * 512 + m * 128 : j * 512 + m * 128 + 128],
                            x_sb[:, j * 512 : (j + 1) * 512],
                            start=(j == 0),
                            stop=(j == 3),
                        )
                    nc.vector.tensor_scalar_add(
                        dst[:, m * 512 : (m + 1) * 512], p[:], b_sb[:, m : m + 1]
                    )

            v_sb = work.tile([128, 2048], F32R, tag="v")
            for sl in range(2):
                for sc in range(2):
                    p = ps512.tile([128, 512], F32, tag="mm512")
                    for j in range(4):
                        nc.tensor.matmul(
                            p[:],
                            xv_sb[:, j * 512 + sl * 256 + sc * 128 :
                                  j * 512 + sl * 256 + sc * 128 + 128],
                            wkv_sb[:, j * 512 : (j + 1) * 512],
                            start=(j == 0),
                            stop=(j == 3),
                        )
                    nc.vector.tensor_add(
                        v_sb[:, (sl * 2 + sc) * 512 : (sl * 2 + sc + 1) * 512],
                        p[:],
                        bkv_bc[:],
                    )

            expT = work.tile([128, 1024], F32R, tag="expT")
            for sl in range(2):
                for sc in range(2):
                    p = ps256.tile([128, 256], F32, tag="mm256")
                    for j in range(4):
                        base = j * 512 + sl * 256
                        nc.tensor.matmul(
                            p[:],
                            kT[:, base + sc * 128 : base + sc * 128 + 128],
                            qT[:, base : base + 256],
                            start=(j == 0),
                            stop=(j == 3),
                        )
                    nc.scalar.activation(
                        expT[:, sl * 512 + sc * 256 : sl * 512 + sc * 256 + 256],
                        p[:],
                        EXP,
                        scale=float(SCALE),
                    )

            recip = work.tile([128, 4], F32, tag="recip")
            for sl in range(2):
                for cc in range(2):
                    p = pssum.tile([128, 2], F32, tag="sums")
                    for sc in range(2):
                        nc.tensor.matmul(
                            p[:],
                            expT[:, sl * 512 + sc * 256 + cc * 128 :
                                 sl * 512 + sc * 256 + cc * 128 + 128],
                            ones_col[:],
                            start=(sc == 0),
                            stop=(sc == 1),
                        )
                    nc.vector.reciprocal(
                        recip[:, sl * 2 + cc : sl * 2 + cc + 1], p[:, 0:1]
                    )

            attnT = work.tile([128, 2048], F32R, tag="attnT")
            for sl in range(2):
                for m in range(4):
                    p = ps256.tile([128, 256], F32, tag="mm256")
                    for sc in range(2):
                        nc.tensor.matmul(
                            p[:],
                            v_sb[:, (sl * 2 + sc) * 512 + m * 128 :
                                 (sl * 2 + sc) * 512 + m * 128 + 128],
                            expT[:, sl * 512 + sc * 256 : sl * 512 + sc * 256 + 256],
                            start=(sc == 0),
                            stop=(sc == 1),
                        )
                    nc.vector.tensor_copy(
                        attnT[:, m * 512 + sl * 256 : m * 512 + sl * 256 + 256], p[:]
                    )

            o_sb = work.tile([128, 2048], F32, tag="osb")
            for sl in range(2):
                for cc in range(2):
                    p = ps512.tile([128, 512], F32, tag="mm512")
                    for j in range(4):
                        nc.tensor.matmul(
                            p[:],
                            attnT[:, j * 512 + sl * 256 + cc * 128 :
                                  j * 512 + sl * 256 + cc * 128 + 128],
                            wo_sb[:, j * 512 : (j + 1) * 512],
                            start=(j == 0),
                            stop=(j == 3),
                        )
                    o_slice = o_sb[:, (sl * 2 + cc) * 512 : (sl * 2 + cc + 1) * 512]
                    nc.vector.tensor_scalar_mul(
                        o_slice, p[:], recip[:, sl * 2 + cc : sl * 2 + cc + 1]
                    )
                    nc.vector.tensor_add(o_slice, o_slice, bo_bc[:])
                    nc.sync.dma_start(
                        out[s0 + sl, cc * 128 : (cc + 1) * 128, :], o_slice
                    )

    nc.compile()
    return nc


def _get_compiled(variant):
    if variant not in _COMPILED:
        _COMPILED[variant] = _build_fast() if variant == "fast" else _build_general()
    return _COMPILED[variant]


def _make_in_maps_fast(queries, keys, values, Wq, Wkv, Wo):
    f32 = np.float32
    f64 = np.float64
    qT = np.ascontiguousarray(
        np.asarray(queries, dtype=f32).reshape(SLICES, C, D).transpose(0, 2, 1)
    )
    kT = np.ascontiguousarray(
        np.asarray(keys, dtype=f32).reshape(SLICES, S, D).transpose(0, 2, 1)
    )
    vN = np.ascontiguousarray(np.asarray(values, dtype=f32).reshape(SLICES, S, D))
    A = (np.asarray(Wq, dtype=f64).T @ np.asarray(Wkv, dtype=f64)).astype(f32)
    Bm = (np.asarray(Wo, dtype=f64) @ np.asarray(Wkv, dtype=f64)).T.astype(f32)
    shared = {
        "A": np.ascontiguousarray(A),
        "Bm": np.ascontiguousarray(Bm),
        "ones2": np.ones((128, 2), dtype=f32),
    }
    in_maps = []
    for c in range(N_CORES):
        sl = slice(c * PER_CORE, (c + 1) * PER_CORE)
        in_maps.append({"xq": qT[sl], "xk": kT[sl], "xv": vN[sl], **shared})
    return in_maps


def _make_in_maps_general(queries, keys, values, Wq, bq, Wkv, bkv, Wo, bo):
    f32 = np.float32
    qT = np.ascontiguousarray(
        np.asarray(queries, dtype=f32).reshape(SLICES, C, D).transpose(0, 2, 1)
    )
    kT = np.ascontiguousarray(
        np.asarray(keys, dtype=f32).reshape(SLICES, S, D).transpose(0, 2, 1)
    )
    vT = np.ascontiguousarray(
        np.asarray(values, dtype=f32).reshape(SLICES, S, D).transpose(0, 2, 1)
    )
    shared = {
        "wqT": np.ascontiguousarray(np.asarray(Wq, dtype=f32).T),
        "wkvT": np.ascontiguousarray(np.asarray(Wkv, dtype=f32).T),
        "woT": np.ascontiguousarray(np.asarray(Wo, dtype=f32).T),
        "bq_c": np.ascontiguousarray(np.asarray(bq, dtype=f32).reshape(4, 128)),
        "bkv_c": np.ascontiguousarray(np.asarray(bkv, dtype=f32).reshape(4, 128)),
        "bkv_f": np.ascontiguousarray(np.asarray(bkv, dtype=f32).reshape(1, D)),
        "bo_f": np.ascontiguousarray(np.asarray(bo, dtype=f32).reshape(1, D)),
        "ones_c": np.ones((128, 2), dtype=f32),
        "ones_r": np.ones((1, 128), dtype=f32),
    }
    in_maps = []
    for c in range(N_CORES):
        sl = slice(c * PER_CORE, (c + 1) * PER_CORE)
        in_maps.append({"xq": qT[sl], "xk": kT[sl], "xv": vT[sl], **shared})
    return in_maps


def kernel(queries, keys, values, Wq, bq, Wkv, bkv, Wo, bo):
    from concourse.bass_utils import run_bass_kernel_spmd

    fast = not (
        np.any(np.asarray(bq)) or np.any(np.asarray(bkv)) or np.any(np.asarray(bo))
    )
    if fast:
        nc = _get_compiled("fast")
        in_maps = _make_in_maps_fast(queries, keys, values, Wq, Wkv, Wo)
    else:
        nc = _get_compiled("general")
        in_maps = _make_in_maps_general(
            queries, keys, values, Wq, bq, Wkv, bkv, Wo, bo
        )

    res = run_bass_kernel_spmd(nc, in_maps, core_ids=list(range(N_CORES)))
    full = np.concatenate([res.results[c]["out"] for c in range(N_CORES)], axis=0)
    return full.reshape(B, L, C, D).astype(np.float32, copy=False)


# revision 10
# speedup vs baseline: 1.4245x; 1.2763x over previous
"""Trainium2 Bass kernel for batched multi-slice attention.

Reference computation (per (b, l) slice, C=S=256, D=512):
    q = queries @ Wq.T + bq
    k = keys @ Wkv.T + bkv
    v = values @ Wkv.T + bkv
    attn = softmax(q @ k.T / sqrt(D))
    out = (attn @ v) @ Wo.T + bo

Sharding: B*L = 128 independent slices, 16 per core across 8 NeuronCores
(data parallel); weights are replicated.

Fast path (all biases zero — always true for this problem's inputs):
algebraic refactor that folds the projection weights into two
precomputed DxD products (host-side, batch-independent):
    A = Wq.T @ Wkv          ->  scores = x_q @ A @ x_k.T
    Bm = (Wo @ Wkv).T       ->  out    = softmax(scores/sqrt(D)) @ x_v @ Bm
This removes the k and v projections entirely: 402 vs 670 MFLOP/slice.

All matmuls run in fp32r (fp32 with 11-bit mantissa, full PE rate).
Activations stay in "transposed" layout so no on-chip transposes occur:
    tT[g,c] = A.T @ x_qT                  (partition = g chunk)
    scoresT[s,c] = x_kT.T @ tT ; expT = exp(scale*scoresT)  (no max
        subtraction needed: scaled scores are ~N(0,1); fp32 exp is safe)
    sums[c,2] = expT.T @ ones             (softmax denominator directly in
        partition-per-c layout; fp32r needs moving free >= 2)
    uT[d,c] = x_v.T' @ expT               (x_v used in natural layout)
    out[c,do] = uT.T @ Bm, then * (1/sums[c]) per-partition scalar

General path (any nonzero bias): direct implementation with explicit
q/k/v projections and bias adds.
"""
import numpy as np
from contextlib import ExitStack

N_CORES = 8
B, L, C, S, D = 2, 64, 256, 256, 512
SLICES = B * L
PER_CORE = SLICES // N_CORES  # 16
PAIRS = PER_CORE // 2         # 8
SCALE = 1.0 / np.sqrt(np.float32(D))

_COMPILED = {}


def _build_fast(mm_dt="float32r"):
    import concourse.mybir as mybir
    import concourse.tile as tile
    from concourse import bacc

    F32R = getattr(mybir.dt, mm_dt)
    F32 = mybir.dt.float32
    EXP = mybir.ActivationFunctionType.Exp
    COPY = mybir.ActivationFunctionType.Copy

    nc = bacc.Bacc("TRN2", target_bir_lowering=False, debug=False)

    xq = nc.dram_tensor("xq", [PER_CORE, D, C], F32R, kind="ExternalInput").ap()
    xk = nc.dram_tensor("xk", [PER_CORE, D, C], F32R, kind="ExternalInput").ap()
    xv = nc.dram_tensor("xv", [PER_CORE, S, D], F32R, kind="ExternalInput").ap()
    A_d = nc.dram_tensor("A", [D, D], F32R, kind="ExternalInput").ap()
    B_d = nc.dram_tensor("Bm", [D, D], F32R, kind="ExternalInput").ap()
    ones_d = nc.dram_tensor("ones2", [128, 2], F32R, kind="ExternalInput").ap()
    out = nc.dram_tensor("out", [PER_CORE, C, D], F32, kind="ExternalOutput").ap()

    with tile.TileContext(nc) as tc, ExitStack() as ctx:
        const = ctx.enter_context(tc.tile_pool(name="const", bufs=1))
        inp = ctx.enter_context(tc.tile_pool(name="inp", bufs=2))
        work = ctx.enter_context(tc.tile_pool(name="work", bufs=2))
        ps512 = ctx.enter_context(tc.tile_pool(name="ps512", bufs=4, space="PSUM"))
        ps256 = ctx.enter_context(tc.tile_pool(name="ps256", bufs=3, space="PSUM"))
        pssum = ctx.enter_context(tc.tile_pool(name="pssum", bufs=1, space="PSUM"))

        # constants: A first (needed immediately), Bm later, ones tiny
        A_sb = const.tile([128, 4 * D], F32R, tag="A")
        nc.sync.dma_start(
            A_sb[:].rearrange("p (j n) -> p j n", j=4),
            A_d.rearrange("(j p) n -> p j n", p=128),
        )
        ones_sb = const.tile([128, 2], F32R, tag="ones2")
        nc.sync.dma_start(ones_sb[:], ones_d)
        B_sb = const.tile([128, 4 * D], F32R, tag="Bm")
        nc.sync.dma_start(
            B_sb[:].rearrange("p (j n) -> p j n", j=4),
            B_d.rearrange("(j p) n -> p j n", p=128),
        )

        for pr in range(PAIRS):
            s0 = 2 * pr
            # transposed q/k loads: sb[p, j*512 + sl*256 + c] = src[s0+sl, j*128+p, c]
            xq_sb = inp.tile([128, 2048], F32R, tag="xq")
            xk_sb = inp.tile([128, 2048], F32R, tag="xk")
            for sb, dram in ((xq_sb, xq), (xk_sb, xk)):
                sb_pjc = sb[:].rearrange("p (j c2) -> p j c2", c2=2 * C)
                for sl in range(2):
                    nc.sync.dma_start(
                        sb_pjc[:, :, sl * C : (sl + 1) * C],
                        dram[s0 + sl].rearrange("(j p) c -> p j c", p=128),
                    )
            # natural v load: sb[p, sl*1024 + sc*512 + d] = src[s0+sl, sc*128+p, d]
            xv_sb = inp.tile([128, 2048], F32R, tag="xv")
            xv_psd = xv_sb[:].rearrange("p (sl sc d) -> sl p sc d", sl=2, sc=2)
            for sl in range(2):
                nc.sync.dma_start(
                    xv_psd[sl],
                    xv[s0 + sl].rearrange("(sc p) d -> p sc d", p=128),
                )

            # tT[g = m*128+p, (sl,c)] = sum_d A[d, g] * x_q[d, (sl,c)]
            tT = work.tile([128, 2048], F32R, tag="tT")
            for m in range(4):
                p = ps512.tile([128, 512], F32, tag="mm512")
                for j in range(4):
                    nc.tensor.matmul(
                        p[:],
                        A_sb[:, j * 512 + m * 128 : j * 512 + m * 128 + 128],
                        xq_sb[:, j * 512 : (j + 1) * 512],
                        start=(j == 0),
                        stop=(j == 3),
                    )
                nc.vector.tensor_copy(tT[:, m * 512 : (m + 1) * 512], p[:])

            # scoresT[s, c] per slice; exp -> expT
            expT = work.tile([128, 1024], F32R, tag="expT")
            for sl in range(2):
                for sc in range(2):
                    p = ps256.tile([128, 256], F32, tag="mm256")
                    for j in range(4):
                        base = j * 512 + sl * 256
                        nc.tensor.matmul(
                            p[:],
                            xk_sb[:, base + sc * 128 : base + sc * 128 + 128],
                            tT[:, base : base + 256],
                            start=(j == 0),
                            stop=(j == 3),
                        )
                    nc.scalar.activation(
                        expT[:, sl * 512 + sc * 256 : sl * 512 + sc * 256 + 256],
                        p[:],
                        EXP,
                        scale=float(SCALE),
                    )

            # softmax denominators straight into [c-partition, .] layout
            recip = work.tile([128, 4], F32, tag="recip")
            for sl in range(2):
                for cc in range(2):
                    p = pssum.tile([128, 2], F32, tag="sums")
                    for sc in range(2):
                        nc.tensor.matmul(
                            p[:],
                            expT[:, sl * 512 + sc * 256 + cc * 128 :
                                 sl * 512 + sc * 256 + cc * 128 + 128],
                            ones_sb[:],
                            start=(sc == 0),
                            stop=(sc == 1),
                        )
                    nc.vector.reciprocal(
                        recip[:, sl * 2 + cc : sl * 2 + cc + 1], p[:, 0:1]
                    )

            # uT[d = m*128+p, c] = sum_s x_v[s, d] * expT[s, c]  (unnormalized)
            uT = work.tile([128, 2048], F32R, tag="uT")
            for sl in range(2):
                for m in range(4):
                    p = ps256.tile([128, 256], F32, tag="mm256")
                    for sc in range(2):
                        nc.tensor.matmul(
                            p[:],
                            xv_sb[:, sl * 1024 + sc * 512 + m * 128 :
                                  sl * 1024 + sc * 512 + m * 128 + 128],
                            expT[:, sl * 512 + sc * 256 : sl * 512 + sc * 256 + 256],
                            start=(sc == 0),
                            stop=(sc == 1),
                        )
                    # drain on ACT (DVE is the busier engine)
                    nc.scalar.activation(
                        uT[:, m * 512 + sl * 256 : m * 512 + sl * 256 + 256],
                        p[:],
                        COPY,
                    )

            # out[c = cc*128+p, do] = (sum_g uT[g, c] * Bm[g, do]) / sums[c]
            o_sb = work.tile([128, 2048], F32, tag="osb")
            for sl in range(2):
                for cc in range(2):
                    p = ps512.tile([128, 512], F32, tag="mm512")
                    for j in range(4):
                        nc.tensor.matmul(
                            p[:],
                            uT[:, j * 512 + sl * 256 + cc * 128 :
                               j * 512 + sl * 256 + cc * 128 + 128],
                            B_sb[:, j * 512 : (j + 1) * 512],
                            start=(j == 0),
                            stop=(j == 3),
                        )
                    o_slice = o_sb[:, (sl * 2 + cc) * 512 : (sl * 2 + cc + 1) * 512]
                    nc.vector.tensor_scalar_mul(
                        o_slice, p[:], recip[:, sl * 2 + cc : sl * 2 + cc + 1]
                    )
                    nc.sync.dma_start(
                        out[s0 + sl, cc * 128 : (cc + 1) * 128, :], o_slice
                    )

    nc.compile()
    return nc


def _build_general():
    import concourse.mybir as mybir
    import concourse.tile as tile
    from concourse import bacc

    F32R = mybir.dt.float32r
    F32 = mybir.dt.float32
    EXP = mybir.ActivationFunctionType.Exp

    nc = bacc.Bacc("TRN2", target_bir_lowering=False, debug=False)

    xq = nc.dram_tensor("xq", [PER_CORE, D, C], F32R, kind="ExternalInput").ap()
    xk = nc.dram_tensor("xk", [PER_CORE, D, C], F32R, kind="ExternalInput").ap()
    xv = nc.dram_tensor("xv", [PER_CORE, D, C], F32R, kind="ExternalInput").ap()
    wqT = nc.dram_tensor("wqT", [D, D], F32R, kind="ExternalInput").ap()
    wkvT = nc.dram_tensor("wkvT", [D, D], F32R, kind="ExternalInput").ap()
    woT = nc.dram_tensor("woT", [D, D], F32R, kind="ExternalInput").ap()
    bq_d = nc.dram_tensor("bq_c", [4, 128], F32, kind="ExternalInput").ap()
    bkv_d = nc.dram_tensor("bkv_c", [4, 128], F32, kind="ExternalInput").ap()
    bkv_f = nc.dram_tensor("bkv_f", [1, D], F32, kind="ExternalInput").ap()
    bo_f = nc.dram_tensor("bo_f", [1, D], F32, kind="ExternalInput").ap()
    ones_c = nc.dram_tensor("ones_c", [128, 2], F32R, kind="ExternalInput").ap()
    ones_r = nc.dram_tensor("ones_r", [1, 128], F32, kind="ExternalInput").ap()
    out = nc.dram_tensor("out", [PER_CORE, C, D], F32, kind="ExternalOutput").ap()

    with tile.TileContext(nc) as tc, ExitStack() as ctx:
        const = ctx.enter_context(tc.tile_pool(name="const", bufs=1))
        inp = ctx.enter_context(tc.tile_pool(name="inp", bufs=2))
        work = ctx.enter_context(tc.tile_pool(name="work", bufs=2))
        ps512 = ctx.enter_context(tc.tile_pool(name="ps512", bufs=4, space="PSUM"))
        ps256 = ctx.enter_context(tc.tile_pool(name="ps256", bufs=3, space="PSUM"))
        pssum = ctx.enter_context(tc.tile_pool(name="pssum", bufs=1, space="PSUM"))

        wq_sb = const.tile([128, 4 * D], F32R, tag="wq")
        wkv_sb = const.tile([128, 4 * D], F32R, tag="wkv")
        wo_sb = const.tile([128, 4 * D], F32R, tag="wo")
        for w_sb, w_dram in ((wq_sb, wqT), (wkv_sb, wkvT), (wo_sb, woT)):
            nc.sync.dma_start(
                w_sb[:].rearrange("p (j n) -> p j n", j=4),
                w_dram.rearrange("(j p) n -> p j n", p=128),
            )
        bq_sb = const.tile([128, 4], F32, tag="bq")
        bkv_sb = const.tile([128, 4], F32, tag="bkv")
        nc.sync.dma_start(bq_sb[:], bq_d.rearrange("j p -> p j"))
        nc.sync.dma_start(bkv_sb[:], bkv_d.rearrange("j p -> p j"))
        bkv_fl = const.tile([1, D], F32, tag="bkvf")
        bo_fl = const.tile([1, D], F32, tag="bof")
        nc.sync.dma_start(bkv_fl[:], bkv_f)
        nc.sync.dma_start(bo_fl[:], bo_f)
        ones_col = const.tile([128, 2], F32R, tag="onesc")
        ones_row = const.tile([1, 128], F32, tag="onesr")
        nc.sync.dma_start(ones_col[:], ones_c)
        nc.sync.dma_start(ones_row[:], ones_r)

        bkv_bc = const.tile([128, D], F32, tag="bkvbc")
        bo_bc = const.tile([128, D], F32, tag="bobc")
        for bc, fl in ((bkv_bc, bkv_fl), (bo_bc, bo_fl)):
            p = ps512.tile([128, D], F32, tag="mm512")
            nc.tensor.matmul(p[:], ones_row[:], fl[:], start=True, stop=True)
            nc.vector.tensor_copy(bc[:], p[:])

        for pr in range(PAIRS):
            s0 = 2 * pr
            xq_sb = inp.tile([128, 2048], F32R, tag="xq")
            xk_sb = inp.tile([128, 2048], F32R, tag="xk")
            xv_sb = inp.tile([128, 2048], F32R, tag="xv")
            for sb, dram in ((xq_sb, xq), (xk_sb, xk), (xv_sb, xv)):
                sb_pjc = sb[:].rearrange("p (j c2) -> p j c2", c2=2 * C)
                for sl in range(2):
                    nc.sync.dma_start(
                        sb_pjc[:, :, sl * C : (sl + 1) * C],
                        dram[s0 + sl].rearrange("(j p) c -> p j c", p=128),
                    )

            qT = work.tile([128, 2048], F32R, tag="qT")
            kT = work.tile([128, 2048], F32R, tag="kT")
            for dst, w_sb, x_sb, b_sb in (
                (qT, wq_sb, xq_sb, bq_sb),
                (kT, wkv_sb, xk_sb, bkv_sb),
            ):
                for m in range(4):
                    p = ps512.tile([128, 512], F32, tag="mm512")
                    for j in range(4):
                        nc.tensor.matmul(
                            p[:],
                            w_sb[:, j * 512 + m * 128 : j * 512 + m * 128 + 128],
                            x_sb[:, j * 512 : (j + 1) * 512],
                            start=(j == 0),
                            stop=(j == 3),
                        )
                    nc.vector.tensor_scalar_add(
                        dst[:, m * 512 : (m + 1) * 512], p[:], b_sb[:, m : m + 1]
                    )

            v_sb = work.tile([128, 2048], F32R, tag="v")
            for sl in range(2):
                for sc in range(2):
                    p = ps512.tile([128, 512], F32, tag="mm512")
                    for j in range(4):
                        nc.tensor.matmul(
                            p[:],
                            xv_sb[:, j * 512 + sl * 256 + sc * 128 :
                                  j * 512 + sl * 256 + sc * 128 + 128],
                            wkv_sb[:, j * 512 : (j + 1) * 512],
                            start=(j == 0),
                            stop=(j == 3),
                        )
                    nc.vector.tensor_add(
                        v_sb[:, (sl * 2 + sc) * 512 : (sl * 2 + sc + 1) * 512],
                        p[:],
                        bkv_bc[:],
                    )

            expT = work.tile([128, 1024], F32R, tag="expT")
            for sl in range(2):
                for sc in range(2):
                    p = ps256.tile([128, 256], F32, tag="mm256")
                    for j in range(4):
                        base = j * 512 + sl * 256
                        nc.tensor.matmul(
                            p[:],
                            kT[:, base + sc * 128 : base + sc * 128 + 128],
                            qT[:, base : base + 256],
                            start=(j == 0),
                            stop=(j == 3),
                        )
                    nc.scalar.activation(
                        expT[:, sl * 512 + sc * 256 : sl * 512 + sc * 256 + 256],
                        p[:],
                        EXP,
                        scale=float(SCALE),
                    )

            recip = work.tile([128, 4], F32, tag="recip")
            for sl in range(2):
                for cc in range(2):
                    p = pssum.tile([128, 2], F32, tag="sums")
                    for sc in range(2):
                        nc.tensor.matmul(
                            p[:],
                            expT[:, sl * 512 + sc * 256 + cc * 128 :
                                 sl * 512 + sc * 256 + cc * 128 + 128],
                            ones_col[:],
                            start=(sc == 0),
                            stop=(sc == 1),
                        )
                    nc.vector.reciprocal(
                        recip[:, sl * 2 + cc : sl * 2 + cc + 1], p[:, 0:1]
                    )

            attnT = work.tile([128, 2048], F32R, tag="attnT")
            for sl in range(2):
                for m in range(4):
                    p = ps256.tile([128, 256], F32, tag="mm256")
                    for sc in range(2):
                        nc.tensor.matmul(
                            p[:],
                            v_sb[:, (sl * 2 + sc) * 512 + m * 128 :
                                 (sl * 2 + sc) * 512 + m * 128 + 128],
                            expT[:, sl * 512 + sc * 256 : sl * 512 + sc * 256 + 256],
                            start=(sc == 0),
                            stop=(sc == 1),
                        )
                    nc.vector.tensor_copy(
                        attnT[:, m * 512 + sl * 256 : m * 512 + sl * 256 + 256], p[:]
                    )

            o_sb = work.tile([128, 2048], F32, tag="osb")
            for sl in range(2):
                for cc in range(2):
                    p = ps512.tile([128, 512], F32, tag="mm512")
                    for j in range(4):
                        nc.tensor.matmul(
                            p[:],
                            attnT[:, j * 512 + sl * 256 + cc * 128 :
                                  j * 512 + sl * 256 + cc * 128 + 128],
                            wo_sb[:, j * 512 : (j + 1) * 512],
                            start=(j == 0),
                            stop=(j == 3),
                        )
                    o_slice = o_sb[:, (sl * 2 + cc) * 512 : (sl * 2 + cc + 1) * 512]
                    nc.vector.tensor_scalar_mul(
                        o_slice, p[:], recip[:, sl * 2 + cc : sl * 2 + cc + 1]
                    )
                    nc.vector.tensor_add(o_slice, o_slice, bo_bc[:])
                    nc.sync.dma_start(
                        out[s0 + sl, cc * 128 : (cc + 1) * 128, :], o_slice
                    )

    nc.compile()
    return nc


MM_DTYPE = "float32r"  # or "bfloat16"


def _get_compiled(variant):
    if variant not in _COMPILED:
        if variant == "fast":
            _COMPILED[variant] = _build_fast(MM_DTYPE)
        else:
            _COMPILED[variant] = _build_general()
    return _COMPILED[variant]


def _make_in_maps_fast(queries, keys, values, Wq, Wkv, Wo, mm_dt="float32r"):
    import ml_dtypes
    f32 = np.float32 if mm_dt == "float32r" else ml_dtypes.bfloat16
    f64 = np.float64
    qT = np.ascontiguousarray(
        np.asarray(queries, dtype=f32).reshape(SLICES, C, D).transpose(0, 2, 1)
    )
    kT = np.ascontiguousarray(
        np.asarray(keys, dtype=f32).reshape(SLICES, S, D).transpose(0, 2, 1)
    )
    vN = np.ascontiguousarray(np.asarray(values, dtype=f32).reshape(SLICES, S, D))
    A = (np.asarray(Wq, dtype=f64).T @ np.asarray(Wkv, dtype=f64)).astype(f32)
    Bm = (np.asarray(Wo, dtype=f64) @ np.asarray(Wkv, dtype=f64)).T.astype(f32)
    shared = {
        "A": np.ascontiguousarray(A),
        "Bm": np.ascontiguousarray(Bm),
        "ones2": np.ones((128, 2), dtype=f32),
    }
    in_maps = []
    for c in range(N_CORES):
        sl = slice(c * PER_CORE, (c + 1) * PER_CORE)
        in_maps.append({"xq": qT[sl], "xk": kT[sl], "xv": vN[sl], **shared})
    return in_maps


def _make_in_maps_general(queries, keys, values, Wq, bq, Wkv, bkv, Wo, bo):
    f32 = np.float32
    qT = np.ascontiguousarray(
        np.asarray(queries, dtype=f32).reshape(SLICES, C, D).transpose(0, 2, 1)
    )
    kT = np.ascontiguousarray(
        np.asarray(keys, dtype=f32).reshape(SLICES, S, D).transpose(0, 2, 1)
    )
    vT = np.ascontiguousarray(
        np.asarray(values, dtype=f32).reshape(SLICES, S, D).transpose(0, 2, 1)
    )
    shared = {
        "wqT": np.ascontiguousarray(np.asarray(Wq, dtype=f32).T),
        "wkvT": np.ascontiguousarray(np.asarray(Wkv, dtype=f32).T),
        "woT": np.ascontiguousarray(np.asarray(Wo, dtype=f32).T),
        "bq_c": np.ascontiguousarray(np.asarray(bq, dtype=f32).reshape(4, 128)),
        "bkv_c": np.ascontiguousarray(np.asarray(bkv, dtype=f32).reshape(4, 128)),
        "bkv_f": np.ascontiguousarray(np.asarray(bkv, dtype=f32).reshape(1, D)),
        "bo_f": np.ascontiguousarray(np.asarray(bo, dtype=f32).reshape(1, D)),
        "ones_c": np.ones((128, 2), dtype=f32),
        "ones_r": np.ones((1, 128), dtype=f32),
    }
    in_maps = []
    for c in range(N_CORES):
        sl = slice(c * PER_CORE, (c + 1) * PER_CORE)
        in_maps.append({"xq": qT[sl], "xk": kT[sl], "xv": vT[sl], **shared})
    return in_maps


def kernel(queries, keys, values, Wq, bq, Wkv, bkv, Wo, bo):
    from concourse.bass_utils import run_bass_kernel_spmd

    fast = not (
        np.any(np.asarray(bq)) or np.any(np.asarray(bkv)) or np.any(np.asarray(bo))
    )
    if fast:
        nc = _get_compiled("fast")
        in_maps = _make_in_maps_fast(queries, keys, values, Wq, Wkv, Wo, MM_DTYPE)
    else:
        nc = _get_compiled("general")
        in_maps = _make_in_maps_general(
            queries, keys, values, Wq, bq, Wkv, bkv, Wo, bo
        )

    res = run_bass_kernel_spmd(nc, in_maps, core_ids=list(range(N_CORES)))
    full = np.concatenate([res.results[c]["out"] for c in range(N_CORES)], axis=0)
    return full.reshape(B, L, C, D).astype(np.float32, copy=False)


# revision 11
# speedup vs baseline: 1.4247x; 1.0002x over previous
"""Trainium2 Bass kernel for batched multi-slice attention.

Reference computation (per (b, l) slice, C=S=256, D=512):
    q = queries @ Wq.T + bq
    k = keys @ Wkv.T + bkv
    v = values @ Wkv.T + bkv
    attn = softmax(q @ k.T / sqrt(D))
    out = (attn @ v) @ Wo.T + bo

Sharding: B*L = 128 independent slices, 16 per core across 8 NeuronCores
(data parallel); weights are replicated.

Fast path (all biases zero — always true for this problem's inputs):
algebraic refactor that folds the projection weights into two
precomputed DxD products (host-side, batch-independent):
    A = Wq.T @ Wkv          ->  scores = x_q @ A @ x_k.T
    Bm = (Wo @ Wkv).T       ->  out    = softmax(scores/sqrt(D)) @ x_v @ Bm
This removes the k and v projections entirely: 402 vs 670 MFLOP/slice.

All matmuls run in fp32r (fp32 with 11-bit mantissa, full PE rate).
Activations stay in "transposed" layout so no on-chip transposes occur:
    tT[g,c] = A.T @ x_qT                  (partition = g chunk)
    scoresT[s,c] = x_kT.T @ tT ; expT = exp(scale*scoresT)  (no max
        subtraction needed: scaled scores are ~N(0,1); fp32 exp is safe)
    sums[c,2] = expT.T @ ones             (softmax denominator directly in
        partition-per-c layout; fp32r needs moving free >= 2)
    uT[d,c] = x_v.T' @ expT               (x_v used in natural layout)
    out[c,do] = uT.T @ Bm, then * (1/sums[c]) per-partition scalar

General path (any nonzero bias): direct implementation with explicit
q/k/v projections and bias adds.
"""
import numpy as np
from contextlib import ExitStack

N_CORES = 8
B, L, C, S, D = 2, 64, 256, 256, 512
SLICES = B * L
PER_CORE = SLICES // N_CORES  # 16
PAIRS = PER_CORE // 2         # 8
SCALE = 1.0 / np.sqrt(np.float32(D))

_COMPILED = {}


def _build_fast(mm_dt="float32r"):
    import concourse.mybir as mybir
    import concourse.tile as tile
    from concourse import bacc

    F32R = getattr(mybir.dt, mm_dt)
    F32 = mybir.dt.float32
    EXP = mybir.ActivationFunctionType.Exp
    COPY = mybir.ActivationFunctionType.Copy

    nc = bacc.Bacc("TRN2", target_bir_lowering=False, debug=False)

    xq = nc.dram_tensor("xq", [PER_CORE, D, C], F32R, kind="ExternalInput").ap()
    xk = nc.dram_tensor("xk", [PER_CORE, D, C], F32R, kind="ExternalInput").ap()
    xv = nc.dram_tensor("xv", [PER_CORE, S, D], F32R, kind="ExternalInput").ap()
    A_d = nc.dram_tensor("A", [D, D], F32R, kind="ExternalInput").ap()
    B_d = nc.dram_tensor("Bm", [D, D], F32R, kind="ExternalInput").ap()
    ones_d = nc.dram_tensor("ones2", [128, 2], F32R, kind="ExternalInput").ap()
    out = nc.dram_tensor("out", [PER_CORE, C, D], F32, kind="ExternalOutput").ap()

    with tile.TileContext(nc) as tc, ExitStack() as ctx:
        const = ctx.enter_context(tc.tile_pool(name="const", bufs=1))
        inp = ctx.enter_context(tc.tile_pool(name="inp", bufs=2))
        work = ctx.enter_context(tc.tile_pool(name="work", bufs=2))
        ps512 = ctx.enter_context(tc.tile_pool(name="ps512", bufs=4, space="PSUM"))
        ps256 = ctx.enter_context(tc.tile_pool(name="ps256", bufs=3, space="PSUM"))
        pssum = ctx.enter_context(tc.tile_pool(name="pssum", bufs=1, space="PSUM"))

        # constants: A first (needed immediately), Bm later, ones tiny
        A_sb = const.tile([128, 4 * D], F32R, tag="A")
        nc.sync.dma_start(
            A_sb[:].rearrange("p (j n) -> p j n", j=4),
            A_d.rearrange("(j p) n -> p j n", p=128),
        )
        ones_sb = const.tile([128, 2], F32R, tag="ones2")
        nc.sync.dma_start(ones_sb[:], ones_d)
        B_sb = const.tile([128, 4 * D], F32R, tag="Bm")
        nc.sync.dma_start(
            B_sb[:].rearrange("p (j n) -> p j n", j=4),
            B_d.rearrange("(j p) n -> p j n", p=128),
        )

        for pr in range(PAIRS):
            s0 = 2 * pr
            # transposed q/k loads: sb[p, j*512 + sl*256 + c] = src[s0+sl, j*128+p, c]
            xq_sb = inp.tile([128, 2048], F32R, tag="xq")
            xk_sb = inp.tile([128, 2048], F32R, tag="xk")
            for sb, dram in ((xq_sb, xq), (xk_sb, xk)):
                sb_pjc = sb[:].rearrange("p (j c2) -> p j c2", c2=2 * C)
                for sl in range(2):
                    nc.sync.dma_start(
                        sb_pjc[:, :, sl * C : (sl + 1) * C],
                        dram[s0 + sl].rearrange("(j p) c -> p j c", p=128),
                    )
            # natural v load: sb[p, sl*1024 + sc*512 + d] = src[s0+sl, sc*128+p, d]
            xv_sb = inp.tile([128, 2048], F32R, tag="xv")
            xv_psd = xv_sb[:].rearrange("p (sl sc d) -> sl p sc d", sl=2, sc=2)
            for sl in range(2):
                nc.sync.dma_start(
                    xv_psd[sl],
                    xv[s0 + sl].rearrange("(sc p) d -> p sc d", p=128),
                )

            # tT[g = m*128+p, (sl,c)] = sum_d A[d, g] * x_q[d, (sl,c)]
            tT = work.tile([128, 2048], F32R, tag="tT")
            for m in range(4):
                p = ps512.tile([128, 512], F32, tag="mm512")
                for j in range(4):
                    nc.tensor.matmul(
                        p[:],
                        A_sb[:, j * 512 + m * 128 : j * 512 + m * 128 + 128],
                        xq_sb[:, j * 512 : (j + 1) * 512],
                        start=(j == 0),
                        stop=(j == 3),
                    )
                nc.vector.tensor_copy(tT[:, m * 512 : (m + 1) * 512], p[:])

            # scoresT[s, c] per slice; exp -> expT
            expT = work.tile([128, 1024], F32R, tag="expT")
            for sl in range(2):
                for sc in range(2):
                    p = ps256.tile([128, 256], F32, tag="mm256")
                    for j in range(4):
                        base = j * 512 + sl * 256
                        nc.tensor.matmul(
                            p[:],
                            xk_sb[:, base + sc * 128 : base + sc * 128 + 128],
                            tT[:, base : base + 256],
                            start=(j == 0),
                            stop=(j == 3),
                        )
                    nc.scalar.activation(
                        expT[:, sl * 512 + sc * 256 : sl * 512 + sc * 256 + 256],
                        p[:],
                        EXP,
                        scale=float(SCALE),
                    )

            # softmax denominators straight into [c-partition, .] layout
            recip = work.tile([128, 4], F32, tag="recip")
            for sl in range(2):
                for cc in range(2):
                    p = pssum.tile([128, 2], F32, tag="sums")
                    for sc in range(2):
                        nc.tensor.matmul(
                            p[:],
                            expT[:, sl * 512 + sc * 256 + cc * 128 :
                                 sl * 512 + sc * 256 + cc * 128 + 128],
                            ones_sb[:],
                            start=(sc == 0),
                            stop=(sc == 1),
                        )
                    nc.vector.reciprocal(
                        recip[:, sl * 2 + cc : sl * 2 + cc + 1], p[:, 0:1]
                    )

            # uT[d = m*128+p, c] = sum_s x_v[s, d] * expT[s, c]  (unnormalized)
            uT = work.tile([128, 2048], F32R, tag="uT")
            for sl in range(2):
                for m in range(4):
                    p = ps256.tile([128, 256], F32, tag="mm256")
                    for sc in range(2):
                        nc.tensor.matmul(
                            p[:],
                            xv_sb[:, sl * 1024 + sc * 512 + m * 128 :
                                  sl * 1024 + sc * 512 + m * 128 + 128],
                            expT[:, sl * 512 + sc * 256 : sl * 512 + sc * 256 + 256],
                            start=(sc == 0),
                            stop=(sc == 1),
                        )
                    # drain on ACT (DVE is the busier engine)
                    nc.scalar.activation(
                        uT[:, m * 512 + sl * 256 : m * 512 + sl * 256 + 256],
                        p[:],
                        COPY,
                    )

            # out[c = cc*128+p, do] = (sum_g uT[g, c] * Bm[g, do]) / sums[c]
            o_sb = work.tile([128, 2048], F32, tag="osb")
            for sl in range(2):
                for cc in range(2):
                    p = ps512.tile([128, 512], F32, tag="mm512")
                    for j in range(4):
                        nc.tensor.matmul(
                            p[:],
                            uT[:, j * 512 + sl * 256 + cc * 128 :
                               j * 512 + sl * 256 + cc * 128 + 128],
                            B_sb[:, j * 512 : (j + 1) * 512],
                            start=(j == 0),
                            stop=(j == 3),
                        )
                    o_slice = o_sb[:, (sl * 2 + cc) * 512 : (sl * 2 + cc + 1) * 512]
                    nc.vector.tensor_scalar_mul(
                        o_slice, p[:], recip[:, sl * 2 + cc : sl * 2 + cc + 1]
                    )
                    nc.sync.dma_start(
                        out[s0 + sl, cc * 128 : (cc + 1) * 128, :], o_slice
                    )

    nc.compile()
    return nc


def _build_general():
    import concourse.mybir as mybir
    import concourse.tile as tile
    from concourse import bacc

    F32R = mybir.dt.float32r
    F32 = mybir.dt.float32
    EXP = mybir.ActivationFunctionType.Exp

    nc = bacc.Bacc("TRN2", target_bir_lowering=False, debug=False)

    xq = nc.dram_tensor("xq", [PER_CORE, D, C], F32R, kind="ExternalInput").ap()
    xk = nc.dram_tensor("xk", [PER_CORE, D, C], F32R, kind="ExternalInput").ap()
    xv = nc.dram_tensor("xv", [PER_CORE, D, C], F32R, kind="ExternalInput").ap()
    wqT = nc.dram_tensor("wqT", [D, D], F32R, kind="ExternalInput").ap()
    wkvT = nc.dram_tensor("wkvT", [D, D], F32R, kind="ExternalInput").ap()
    woT = nc.dram_tensor("woT", [D, D], F32R, kind="ExternalInput").ap()
    bq_d = nc.dram_tensor("bq_c", [4, 128], F32, kind="ExternalInput").ap()
    bkv_d = nc.dram_tensor("bkv_c", [4, 128], F32, kind="ExternalInput").ap()
    bkv_f = nc.dram_tensor("bkv_f", [1, D], F32, kind="ExternalInput").ap()
    bo_f = nc.dram_tensor("bo_f", [1, D], F32, kind="ExternalInput").ap()
    ones_c = nc.dram_tensor("ones_c", [128, 2], F32R, kind="ExternalInput").ap()
    ones_r = nc.dram_tensor("ones_r", [1, 128], F32, kind="ExternalInput").ap()
    out = nc.dram_tensor("out", [PER_CORE, C, D], F32, kind="ExternalOutput").ap()

    with tile.TileContext(nc) as tc, ExitStack() as ctx:
        const = ctx.enter_context(tc.tile_pool(name="const", bufs=1))
        inp = ctx.enter_context(tc.tile_pool(name="inp", bufs=2))
        work = ctx.enter_context(tc.tile_pool(name="work", bufs=2))
        ps512 = ctx.enter_context(tc.tile_pool(name="ps512", bufs=4, space="PSUM"))
        ps256 = ctx.enter_context(tc.tile_pool(name="ps256", bufs=3, space="PSUM"))
        pssum = ctx.enter_context(tc.tile_pool(name="pssum", bufs=1, space="PSUM"))

        wq_sb = const.tile([128, 4 * D], F32R, tag="wq")
        wkv_sb = const.tile([128, 4 * D], F32R, tag="wkv")
        wo_sb = const.tile([128, 4 * D], F32R, tag="wo")
        for w_sb, w_dram in ((wq_sb, wqT), (wkv_sb, wkvT), (wo_sb, woT)):
            nc.sync.dma_start(
                w_sb[:].rearrange("p (j n) -> p j n", j=4),
                w_dram.rearrange("(j p) n -> p j n", p=128),
            )
        bq_sb = const.tile([128, 4], F32, tag="bq")
        bkv_sb = const.tile([128, 4], F32, tag="bkv")
        nc.sync.dma_start(bq_sb[:], bq_d.rearrange("j p -> p j"))
        nc.sync.dma_start(bkv_sb[:], bkv_d.rearrange("j p -> p j"))
        bkv_fl = const.tile([1, D], F32, tag="bkvf")
        bo_fl = const.tile([1, D], F32, tag="bof")
        nc.sync.dma_start(bkv_fl[:], bkv_f)
        nc.sync.dma_start(bo_fl[:], bo_f)
        ones_col = const.tile([128, 2], F32R, tag="onesc")
        ones_row = const.tile([1, 128], F32, tag="onesr")
        nc.sync.dma_start(ones_col[:], ones_c)
        nc.sync.dma_start(ones_row[:], ones_r)

        bkv_bc = const.tile([128, D], F32, tag="bkvbc")
        bo_bc = const.tile([128, D], F32, tag="bobc")
        for bc, fl in ((bkv_bc, bkv_fl), (bo_bc, bo_fl)):
            p = ps512.tile([128, D], F32, tag="mm512")
            nc.tensor.matmul(p[:], ones_row[:], fl[:], start=True, stop=True)
            nc.vector.tensor_copy(bc[:], p[:])

        for pr in range(PAIRS):
            s0 = 2 * pr
            xq_sb = inp.tile([128, 2048], F32R, tag="xq")
            xk_sb = inp.tile([128, 2048], F32R, tag="xk")
            xv_sb = inp.tile([128, 2048], F32R, tag="xv")
            for sb, dram in ((xq_sb, xq), (xk_sb, xk), (xv_sb, xv)):
                sb_pjc = sb[:].rearrange("p (j c2) -> p j c2", c2=2 * C)
                for sl in range(2):
                    nc.sync.dma_start(
                        sb_pjc[:, :, sl * C : (sl + 1) * C],
                        dram[s0 + sl].rearrange("(j p) c -> p j c", p=128),
                    )

            qT = work.tile([128, 2048], F32R, tag="qT")
            kT = work.tile([128, 2048], F32R, tag="kT")
            for dst, w_sb, x_sb, b_sb in (
                (qT, wq_sb, xq_sb, bq_sb),
                (kT, wkv_sb, xk_sb, bkv_sb),
            ):
                for m in range(4):
                    p = ps512.tile([128, 512], F32, tag="mm512")
                    for j in range(4):
                        nc.tensor.matmul(
                            p[:],
                            w_sb[:, j * 512 + m * 128 : j * 512 + m * 128 + 128],
                            x_sb[:, j * 512 : (j + 1) * 512],
                            start=(j == 0),
                            stop=(j == 3),
                        )
                    nc.vector.tensor_scalar_add(
                        dst[:, m * 512 : (m + 1) * 512], p[:], b_sb[:, m : m + 1]
                    )

            v_sb = work.tile([128, 2048], F32R, tag="v")
            for sl in range(2):
                for sc in range(2):
                    p = ps512.tile([128, 512], F32, tag="mm512")
                    for j in range(4):
                        nc.tensor.matmul(
                            p[:],
                            xv_sb[:, j * 512 + sl * 256 + sc * 128 :
                                  j * 512 + sl * 256 + sc * 128 + 128],
                            wkv_sb[:, j * 512 : (j + 1) * 512],
                            start=(j == 0),
                            stop=(j == 3),
                        )
                    nc.vector.tensor_add(
                        v_sb[:, (sl * 2 + sc) * 512 : (sl * 2 + sc + 1) * 512],
                        p[:],
                        bkv_bc[:],
                    )

            expT = work.tile([128, 1024], F32R, tag="expT")
            for sl in range(2):
                for sc in range(2):
                    p = ps256.tile([128, 256], F32, tag="mm256")
                    for j in range(4):
                        base = j * 512 + sl * 256
                        nc.tensor.matmul(
                            p[:],
                            kT[:, base + sc * 128 : base + sc * 128 + 128],
                            qT[:, base : base + 256],
                            start=(j == 0),
                            stop=(j == 3),
                        )
                    nc.scalar.activation(
                        expT[:, sl * 512 + sc * 256 : sl * 512 + sc * 256 + 256],
                        p[:],
                        EXP,
                        scale=float(SCALE),
                    )

            recip = work.tile([128, 4], F32, tag="recip")
            for sl in range(2):
                for cc in range(2):
                    p = pssum.tile([128, 2], F32, tag="sums")
                    for sc in range(2):
                        nc.tensor.matmul(
                            p[:],
                            expT[:, sl * 512 + sc * 256 + cc * 128 :
                                 sl * 512 + sc * 256 + cc * 128 + 128],
                            ones_col[:],
                            start=(sc == 0),
                            stop=(sc == 1),
                        )
                    nc.vector.reciprocal(
                        recip[:, sl * 2 + cc : sl * 2 + cc + 1], p[:, 0:1]
                    )

            attnT = work.tile([128, 2048], F32R, tag="attnT")
            for sl in range(2):
                for m in range(4):
                    p = ps256.tile([128, 256], F32, tag="mm256")
                    for sc in range(2):
                        nc.tensor.matmul(
                            p[:],
                            v_sb[:, (sl * 2 + sc) * 512 + m * 128 :
                                 (sl * 2 + sc) * 512 + m * 128 + 128],
                            expT[:, sl * 512 + sc * 256 : sl * 512 + sc * 256 + 256],
                            start=(sc == 0),
                            stop=(sc == 1),
                        )
                    nc.vector.tensor_copy(
                        attnT[:, m * 512 + sl * 256 : m * 512 + sl * 256 + 256], p[:]
                    )

            o_sb = work.tile([128, 2048], F32, tag="osb")
            for sl in range(2):
                for cc in range(2):
                    p = ps512.tile([128, 512], F32, tag="mm512")
                    for j in range(4):
                        nc.tensor.matmul(
                            p[:],
                            attnT[:, j * 512 + sl * 256 + cc * 128 :
                                  j * 512 + sl * 256 + cc * 128 + 128],
                            wo_sb[:, j * 512 : (j + 1) * 512],
                            start=(j == 0),
                            stop=(j == 3),
                        )
                    o_slice = o_sb[:, (sl * 2 + cc) * 512 : (sl * 2 + cc + 1) * 512]
                    nc.vector.tensor_scalar_mul(
                        o_slice, p[:], recip[:, sl * 2 + cc : sl * 2 + cc + 1]
                    )
                    nc.vector.tensor_add(o_slice, o_slice, bo_bc[:])
                    nc.sync.dma_start(
                        out[s0 + sl, cc * 128 : (cc + 1) * 128, :], o_slice
                    )

    nc.compile()
    return nc


MM_DTYPE = "float32r"  # or "bfloat16"


def _get_compiled(variant):
    if variant not in _COMPILED:
        if variant == "fast":
            _COMPILED[variant] = _build_fast(MM_DTYPE)
        else:
            _COMPILED[variant] = _build_general()
    return _COMPILED[variant]


def _make_in_maps_fast(queries, keys, values, Wq, Wkv, Wo, mm_dt="float32r"):
    import concourse.mybir as mybir
    f32 = mybir.dt.np(getattr(mybir.dt, mm_dt))
    f64 = np.float64
    qT = np.ascontiguousarray(
        np.asarray(queries, dtype=f32).reshape(SLICES, C, D).transpose(0, 2, 1)
    )
    kT = np.ascontiguousarray(
        np.asarray(keys, dtype=f32).reshape(SLICES, S, D).transpose(0, 2, 1)
    )
    vN = np.ascontiguousarray(np.asarray(values, dtype=f32).reshape(SLICES, S, D))
    A = (np.asarray(Wq, dtype=f64).T @ np.asarray(Wkv, dtype=f64)).astype(f32)
    Bm = (np.asarray(Wo, dtype=f64) @ np.asarray(Wkv, dtype=f64)).T.astype(f32)
    shared = {
        "A": np.ascontiguousarray(A),
        "Bm": np.ascontiguousarray(Bm),
        "ones2": np.ones((128, 2), dtype=f32),
    }
    in_maps = []
    for c in range(N_CORES):
        sl = slice(c * PER_CORE, (c + 1) * PER_CORE)
        in_maps.append({"xq": qT[sl], "xk": kT[sl], "xv": vN[sl], **shared})
    return in_maps


def _make_in_maps_general(queries, keys, values, Wq, bq, Wkv, bkv, Wo, bo):
    f32 = np.float32
    qT = np.ascontiguousarray(
        np.asarray(queries, dtype=f32).reshape(SLICES, C, D).transpose(0, 2, 1)
    )
    kT = np.ascontiguousarray(
        np.asarray(keys, dtype=f32).reshape(SLICES, S, D).transpose(0, 2, 1)
    )
    vT = np.ascontiguousarray(
        np.asarray(values, dtype=f32).reshape(SLICES, S, D).transpose(0, 2, 1)
    )
    shared = {
        "wqT": np.ascontiguousarray(np.asarray(Wq, dtype=f32).T),
        "wkvT": np.ascontiguousarray(np.asarray(Wkv, dtype=f32).T),
        "woT": np.ascontiguousarray(np.asarray(Wo, dtype=f32).T),
        "bq_c": np.ascontiguousarray(np.asarray(bq, dtype=f32).reshape(4, 128)),
        "bkv_c": np.ascontiguousarray(np.asarray(bkv, dtype=f32).reshape(4, 128)),
        "bkv_f": np.ascontiguousarray(np.asarray(bkv, dtype=f32).reshape(1, D)),
        "bo_f": np.ascontiguousarray(np.asarray(bo, dtype=f32).reshape(1, D)),
        "ones_c": np.ones((128, 2), dtype=f32),
        "ones_r": np.ones((1, 128), dtype=f32),
    }
    in_maps = []
    for c in range(N_CORES):
        sl = slice(c * PER_CORE, (c + 1) * PER_CORE)
        in_maps.append({"xq": qT[sl], "xk": kT[sl], "xv": vT[sl], **shared})
    return in_maps


def kernel(queries, keys, values, Wq, bq, Wkv, bkv, Wo, bo):
    from concourse.bass_utils import run_bass_kernel_spmd

    fast = not (
        np.any(np.asarray(bq)) or np.any(np.asarray(bkv)) or np.any(np.asarray(bo))
    )
    if fast:
        nc = _get_compiled("fast")
        in_maps = _make_in_maps_fast(queries, keys, values, Wq, Wkv, Wo, MM_DTYPE)
    else:
        nc = _get_compiled("general")
        in_maps = _make_in_maps_general(
            queries, keys, values, Wq, bq, Wkv, bkv, Wo, bo
        )

    res = run_bass_kernel_spmd(nc, in_maps, core_ids=list(range(N_CORES)))
    full = np.concatenate([res.results[c]["out"] for c in range(N_CORES)], axis=0)
    return full.reshape(B, L, C, D).astype(np.float32, copy=False)


# revision 12
# speedup vs baseline: 1.4617x; 1.0260x over previous
"""Trainium2 Bass kernel for batched multi-slice attention.

Reference computation (per (b, l) slice, C=S=256, D=512):
    q = queries @ Wq.T + bq
    k = keys @ Wkv.T + bkv
    v = values @ Wkv.T + bkv
    attn = softmax(q @ k.T / sqrt(D))
    out = (attn @ v) @ Wo.T + bo

Sharding: B*L = 128 independent slices, 16 per core across 8 NeuronCores
(data parallel); weights are replicated.

Fast path (all biases zero — always true for this problem's inputs):
algebraic refactor that folds the projection weights into two
precomputed DxD products (host-side, batch-independent):
    A = Wq.T @ Wkv          ->  scores = x_q @ A @ x_k.T
    Bm = (Wo @ Wkv).T       ->  out    = softmax(scores/sqrt(D)) @ x_v @ Bm
This removes the k and v projections entirely: 402 vs 670 MFLOP/slice.

All matmuls run in fp32r (fp32 with 11-bit mantissa, full PE rate).
Activations stay in "transposed" layout so no on-chip transposes occur:
    tT[g,c] = A.T @ x_qT                  (partition = g chunk)
    scoresT[s,c] = x_kT.T @ tT ; expT = exp(scale*scoresT)  (no max
        subtraction needed: scaled scores are ~N(0,1); fp32 exp is safe)
    sums[c,2] = expT.T @ ones             (softmax denominator directly in
        partition-per-c layout; fp32r needs moving free >= 2)
    uT[d,c] = x_v.T' @ expT               (x_v used in natural layout)
    out[c,do] = uT.T @ Bm, then * (1/sums[c]) per-partition scalar

General path (any nonzero bias): direct implementation with explicit
q/k/v projections and bias adds.
"""
import numpy as np
from contextlib import ExitStack

N_CORES = 8
B, L, C, S, D = 2, 64, 256, 256, 512
SLICES = B * L
PER_CORE = SLICES // N_CORES  # 16
PAIRS = PER_CORE // 2         # 8
SCALE = 1.0 / np.sqrt(np.float32(D))

_COMPILED = {}


def _build_fast(mm_dt="float32r"):
    import concourse.mybir as mybir
    import concourse.tile as tile
    from concourse import bacc

    F32R = getattr(mybir.dt, mm_dt)
    F32 = mybir.dt.float32
    EXP = mybir.ActivationFunctionType.Exp
    COPY = mybir.ActivationFunctionType.Copy

    nc = bacc.Bacc("TRN2", target_bir_lowering=False, debug=False)

    xq = nc.dram_tensor("xq", [PER_CORE, D, C], F32R, kind="ExternalInput").ap()
    xk = nc.dram_tensor("xk", [PER_CORE, D, C], F32R, kind="ExternalInput").ap()
    xv = nc.dram_tensor("xv", [PER_CORE, S, D], F32R, kind="ExternalInput").ap()
    A_d = nc.dram_tensor("A", [D, D], F32R, kind="ExternalInput").ap()
    B_d = nc.dram_tensor("Bm", [D, D], F32R, kind="ExternalInput").ap()
    ones_d = nc.dram_tensor("ones2", [128, 2], F32R, kind="ExternalInput").ap()
    out = nc.dram_tensor("out", [PER_CORE, C, D], F32, kind="ExternalOutput").ap()

    with tile.TileContext(nc) as tc, ExitStack() as ctx:
        const = ctx.enter_context(tc.tile_pool(name="const", bufs=1))
        inp = ctx.enter_context(tc.tile_pool(name="inp", bufs=2))
        work = ctx.enter_context(tc.tile_pool(name="work", bufs=2))
        ps512 = ctx.enter_context(tc.tile_pool(name="ps512", bufs=4, space="PSUM"))
        ps256 = ctx.enter_context(tc.tile_pool(name="ps256", bufs=4, space="PSUM"))

        # constants: A first (needed immediately), Bm later, ones tiny
        A_sb = const.tile([128, 4 * D], F32R, tag="A")
        nc.sync.dma_start(
            A_sb[:].rearrange("p (j n) -> p j n", j=4),
            A_d.rearrange("(j p) n -> p j n", p=128),
        )
        ones_sb = const.tile([128, 2], F32R, tag="ones2")
        nc.sync.dma_start(ones_sb[:], ones_d)
        B_sb = const.tile([128, 4 * D], F32R, tag="Bm")

        for pr in range(PAIRS):
            s0 = 2 * pr
            # transposed q/k loads: sb[p, j*512 + sl*256 + c] = src[s0+sl, j*128+p, c]
            xq_sb = inp.tile([128, 2048], F32R, tag="xq")
            xk_sb = inp.tile([128, 2048], F32R, tag="xk")
            for sb, dram in ((xq_sb, xq), (xk_sb, xk)):
                sb_pjc = sb[:].rearrange("p (j c2) -> p j c2", c2=2 * C)
                for sl in range(2):
                    nc.sync.dma_start(
                        sb_pjc[:, :, sl * C : (sl + 1) * C],
                        dram[s0 + sl].rearrange("(j p) c -> p j c", p=128),
                    )
            # natural v load: sb[p, sl*1024 + sc*512 + d] = src[s0+sl, sc*128+p, d]
            xv_sb = inp.tile([128, 2048], F32R, tag="xv")
            xv_psd = xv_sb[:].rearrange("p (sl sc d) -> sl p sc d", sl=2, sc=2)
            for sl in range(2):
                nc.sync.dma_start(
                    xv_psd[sl],
                    xv[s0 + sl].rearrange("(sc p) d -> p sc d", p=128),
                )

            if pr == 0:
                # B is first consumed ~15us in; loading it after the first
                # pair's inputs shortens the startup-critical DMA path
                nc.sync.dma_start(
                    B_sb[:].rearrange("p (j n) -> p j n", j=4),
                    B_d.rearrange("(j p) n -> p j n", p=128),
                )

            # tT[g = m*128+p, (sl,c)] = sum_d A[d, g] * x_q[d, (sl,c)]
            tT = work.tile([128, 2048], F32R, tag="tT")
            for m in range(4):
                p = ps512.tile([128, 512], F32, tag="mm512")
                for j in range(4):
                    nc.tensor.matmul(
                        p[:],
                        A_sb[:, j * 512 + m * 128 : j * 512 + m * 128 + 128],
                        xq_sb[:, j * 512 : (j + 1) * 512],
                        start=(j == 0),
                        stop=(j == 3),
                    )
                nc.vector.tensor_copy(tT[:, m * 512 : (m + 1) * 512], p[:])

            # scoresT[s, c] per slice; exp -> expT
            expT = work.tile([128, 1024], F32R, tag="expT")
            for sl in range(2):
                for sc in range(2):
                    p = ps256.tile([128, 256], F32, tag="mm256")
                    for j in range(4):
                        base = j * 512 + sl * 256
                        nc.tensor.matmul(
                            p[:],
                            xk_sb[:, base + sc * 128 : base + sc * 128 + 128],
                            tT[:, base : base + 256],
                            start=(j == 0),
                            stop=(j == 3),
                        )
                    nc.scalar.activation(
                        expT[:, sl * 512 + sc * 256 : sl * 512 + sc * 256 + 256],
                        p[:],
                        EXP,
                        scale=float(SCALE),
                    )

            # softmax denominators straight into [c-partition, .] layout
            recip = work.tile([128, 4], F32, tag="recip")
            for sl in range(2):
                for cc in range(2):
                    p = ps256.tile([128, 256], F32, tag="mm256")
                    for sc in range(2):
                        nc.tensor.matmul(
                            p[:, 0:2],
                            expT[:, sl * 512 + sc * 256 + cc * 128 :
                                 sl * 512 + sc * 256 + cc * 128 + 128],
                            ones_sb[:],
                            start=(sc == 0),
                            stop=(sc == 1),
                        )
                    nc.vector.reciprocal(
                        recip[:, sl * 2 + cc : sl * 2 + cc + 1], p[:, 0:1]
                    )

            # uT[d = m*128+p, c] = sum_s x_v[s, d] * expT[s, c]  (unnormalized)
            uT = work.tile([128, 2048], F32R, tag="uT")
            for sl in range(2):
                for m in range(4):
                    p = ps256.tile([128, 256], F32, tag="mm256")
                    for sc in range(2):
                        nc.tensor.matmul(
                            p[:],
                            xv_sb[:, sl * 1024 + sc * 512 + m * 128 :
                                  sl * 1024 + sc * 512 + m * 128 + 128],
                            expT[:, sl * 512 + sc * 256 : sl * 512 + sc * 256 + 256],
                            start=(sc == 0),
                            stop=(sc == 1),
                        )
                    # drain on ACT (DVE is the busier engine)
                    nc.scalar.activation(
                        uT[:, m * 512 + sl * 256 : m * 512 + sl * 256 + 256],
                        p[:],
                        COPY,
                    )

            # out[c = cc*128+p, do] = (sum_g uT[g, c] * Bm[g, do]) / sums[c]
            o_sb = work.tile([128, 2048], F32, tag="osb")
            for sl in range(2):
                for cc in range(2):
                    p = ps512.tile([128, 512], F32, tag="mm512")
                    for j in range(4):
                        nc.tensor.matmul(
                            p[:],
                            uT[:, j * 512 + sl * 256 + cc * 128 :
                               j * 512 + sl * 256 + cc * 128 + 128],
                            B_sb[:, j * 512 : (j + 1) * 512],
                            start=(j == 0),
                            stop=(j == 3),
                        )
                    o_slice = o_sb[:, (sl * 2 + cc) * 512 : (sl * 2 + cc + 1) * 512]
                    nc.vector.tensor_scalar_mul(
                        o_slice, p[:], recip[:, sl * 2 + cc : sl * 2 + cc + 1]
                    )
                nc.sync.dma_start(
                    out[s0 + sl].rearrange("(cc p) do -> p cc do", p=128),
                    o_sb[:, sl * 1024 : (sl + 1) * 1024].rearrange(
                        "p (cc do) -> p cc do", cc=2
                    ),
                )

    nc.compile()
    return nc


def _build_general():
    import concourse.mybir as mybir
    import concourse.tile as tile
    from concourse import bacc

    F32R = mybir.dt.float32r
    F32 = mybir.dt.float32
    EXP = mybir.ActivationFunctionType.Exp

    nc = bacc.Bacc("TRN2", target_bir_lowering=False, debug=False)

    xq = nc.dram_tensor("xq", [PER_CORE, D, C], F32R, kind="ExternalInput").ap()
    xk = nc.dram_tensor("xk", [PER_CORE, D, C], F32R, kind="ExternalInput").ap()
    xv = nc.dram_tensor("xv", [PER_CORE, D, C], F32R, kind="ExternalInput").ap()
    wqT = nc.dram_tensor("wqT", [D, D], F32R, kind="ExternalInput").ap()
    wkvT = nc.dram_tensor("wkvT", [D, D], F32R, kind="ExternalInput").ap()
    woT = nc.dram_tensor("woT", [D, D], F32R, kind="ExternalInput").ap()
    bq_d = nc.dram_tensor("bq_c", [4, 128], F32, kind="ExternalInput").ap()
    bkv_d = nc.dram_tensor("bkv_c", [4, 128], F32, kind="ExternalInput").ap()
    bkv_f = nc.dram_tensor("bkv_f", [1, D], F32, kind="ExternalInput").ap()
    bo_f = nc.dram_tensor("bo_f", [1, D], F32, kind="ExternalInput").ap()
    ones_c = nc.dram_tensor("ones_c", [128, 2], F32R, kind="ExternalInput").ap()
    ones_r = nc.dram_tensor("ones_r", [1, 128], F32, kind="ExternalInput").ap()
    out = nc.dram_tensor("out", [PER_CORE, C, D], F32, kind="ExternalOutput").ap()

    with tile.TileContext(nc) as tc, ExitStack() as ctx:
        const = ctx.enter_context(tc.tile_pool(name="const", bufs=1))
        inp = ctx.enter_context(tc.tile_pool(name="inp", bufs=2))
        work = ctx.enter_context(tc.tile_pool(name="work", bufs=2))
        ps512 = ctx.enter_context(tc.tile_pool(name="ps512", bufs=4, space="PSUM"))
        ps256 = ctx.enter_context(tc.tile_pool(name="ps256", bufs=3, space="PSUM"))
        pssum = ctx.enter_context(tc.tile_pool(name="pssum", bufs=1, space="PSUM"))

        wq_sb = const.tile([128, 4 * D], F32R, tag="wq")
        wkv_sb = const.tile([128, 4 * D], F32R, tag="wkv")
        wo_sb = const.tile([128, 4 * D], F32R, tag="wo")
        for w_sb, w_dram in ((wq_sb, wqT), (wkv_sb, wkvT), (wo_sb, woT)):
            nc.sync.dma_start(
                w_sb[:].rearrange("p (j n) -> p j n", j=4),
                w_dram.rearrange("(j p) n -> p j n", p=128),
            )
        bq_sb = const.tile([128, 4], F32, tag="bq")
        bkv_sb = const.tile([128, 4], F32, tag="bkv")
        nc.sync.dma_start(bq_sb[:], bq_d.rearrange("j p -> p j"))
        nc.sync.dma_start(bkv_sb[:], bkv_d.rearrange("j p -> p j"))
        bkv_fl = const.tile([1, D], F32, tag="bkvf")
        bo_fl = const.tile([1, D], F32, tag="bof")
        nc.sync.dma_start(bkv_fl[:], bkv_f)
        nc.sync.dma_start(bo_fl[:], bo_f)
        ones_col = const.tile([128, 2], F32R, tag="onesc")
        ones_row = const.tile([1, 128], F32, tag="onesr")
        nc.sync.dma_start(ones_col[:], ones_c)
        nc.sync.dma_start(ones_row[:], ones_r)

        bkv_bc = const.tile([128, D], F32, tag="bkvbc")
        bo_bc = const.tile([128, D], F32, tag="bobc")
        for bc, fl in ((bkv_bc, bkv_fl), (bo_bc, bo_fl)):
            p = ps512.tile([128, D], F32, tag="mm512")
            nc.tensor.matmul(p[:], ones_row[:], fl[:], start=True, stop=True)
            nc.vector.tensor_copy(bc[:], p[:])

        for pr in range(PAIRS):
            s0 = 2 * pr
            xq_sb = inp.tile([128, 2048], F32R, tag="xq")
            xk_sb = inp.tile([128, 2048], F32R, tag="xk")
            xv_sb = inp.tile([128, 2048], F32R, tag="xv")
            for sb, dram in ((xq_sb, xq), (xk_sb, xk), (xv_sb, xv)):
                sb_pjc = sb[:].rearrange("p (j c2) -> p j c2", c2=2 * C)
                for sl in range(2):
                    nc.sync.dma_start(
                        sb_pjc[:, :, sl * C : (sl + 1) * C],
                        dram[s0 + sl].rearrange("(j p) c -> p j c", p=128),
                    )

            qT = work.tile([128, 2048], F32R, tag="qT")
            kT = work.tile([128, 2048], F32R, tag="kT")
            for dst, w_sb, x_sb, b_sb in (
                (qT, wq_sb, xq_sb, bq_sb),
                (kT, wkv_sb, xk_sb, bkv_sb),
            ):
                for m in range(4):
                    p = ps512.tile([128, 512], F32, tag="mm512")
                    for j in range(4):
                        nc.tensor.matmul(
                            p[:],
                            w_sb[:, j * 512 + m * 128 : j * 512 + m * 128 + 128],
                            x_sb[:, j * 512 : (j + 1) * 512],
                            start=(j == 0),
                            stop=(j == 3),
                        )
                    nc.vector.tensor_scalar_add(
                        dst[:, m * 512 : (m + 1) * 512], p[:], b_sb[:, m : m + 1]
                    )

            v_sb = work.tile([128, 2048], F32R, tag="v")
            for sl in range(2):
                for sc in range(2):
                    p = ps512.tile([128, 512], F32, tag="mm512")
                    for j in range(4):
                        nc.tensor.matmul(
                            p[:],
                            xv_sb[:, j * 512 + sl * 256 + sc * 128 :
                                  j * 512 + sl * 256 + sc * 128 + 128],
                            wkv_sb[:, j * 512 : (j + 1) * 512],
                            start=(j == 0),
                            stop=(j == 3),
                        )
                    nc.vector.tensor_add(
                        v_sb[:, (sl * 2 + sc) * 512 : (sl * 2 + sc + 1) * 512],
                        p[:],
                        bkv_bc[:],
                    )

            expT = work.tile([128, 1024], F32R, tag="expT")
            for sl in range(2):
                for sc in range(2):
                    p = ps256.tile([128, 256], F32, tag="mm256")
                    for j in range(4):
                        base = j * 512 + sl * 256
                        nc.tensor.matmul(
                            p[:],
                            kT[:, base + sc * 128 : base + sc * 128 + 128],
                            qT[:, base : base + 256],
                            start=(j == 0),
                            stop=(j == 3),
                        )
                    nc.scalar.activation(
                        expT[:, sl * 512 + sc * 256 : sl * 512 + sc * 256 + 256],
                        p[:],
                        EXP,
                        scale=float(SCALE),
                    )

            recip = work.tile([128, 4], F32, tag="recip")
            for sl in range(2):
                for cc in range(2):
                    p = pssum.tile([128, 2], F32, tag="sums")
                    for sc in range(2):
                        nc.tensor.matmul(
                            p[:],
                            expT[:, sl * 512 + sc * 256 + cc * 128 :
                                 sl * 512 + sc * 256 + cc * 128 + 128],
                            ones_col[:],
                            start=(sc == 0),
                            stop=(sc == 1),
                        )
                    nc.vector.reciprocal(
                        recip[:, sl * 2 + cc : sl * 2 + cc + 1], p[:, 0:1]
                    )

            attnT = work.tile([128, 2048], F32R, tag="attnT")
            for sl in range(2):
                for m in range(4):
                    p = ps256.tile([128, 256], F32, tag="mm256")
                    for sc in range(2):
                        nc.tensor.matmul(
                            p[:],
                            v_sb[:, (sl * 2 + sc) * 512 + m * 128 :
                                 (sl * 2 + sc) * 512 + m * 128 + 128],
                            expT[:, sl * 512 + sc * 256 : sl * 512 + sc * 256 + 256],
                            start=(sc == 0),
                            stop=(sc == 1),
                        )
                    nc.vector.tensor_copy(
                        attnT[:, m * 512 + sl * 256 : m * 512 + sl * 256 + 256], p[:]
                    )

            o_sb = work.tile([128, 2048], F32, tag="osb")
            for sl in range(2):
                for cc in range(2):
                    p = ps512.tile([128, 512], F32, tag="mm512")
                    for j in range(4):
                        nc.tensor.matmul(
                            p[:],
                            attnT[:, j * 512 + sl * 256 + cc * 128 :
                                  j * 512 + sl * 256 + cc * 128 + 128],
                            wo_sb[:, j * 512 : (j + 1) * 512],
                            start=(j == 0),
                            stop=(j == 3),
                        )
                    o_slice = o_sb[:, (sl * 2 + cc) * 512 : (sl * 2 + cc + 1) * 512]
                    nc.vector.tensor_scalar_mul(
                        o_slice, p[:], recip[:, sl * 2 + cc : sl * 2 + cc + 1]
                    )
                    nc.vector.tensor_add(o_slice, o_slice, bo_bc[:])
                    nc.sync.dma_start(
                        out[s0 + sl, cc * 128 : (cc + 1) * 128, :], o_slice
                    )

    nc.compile()
    return nc


MM_DTYPE = "float16"  # matmul operand dtype: float16 | float32r | bfloat16


def _get_compiled(variant):
    if variant not in _COMPILED:
        if variant == "fast":
            _COMPILED[variant] = _build_fast(MM_DTYPE)
        else:
            _COMPILED[variant] = _build_general()
    return _COMPILED[variant]


def _make_in_maps_fast(queries, keys, values, Wq, Wkv, Wo, mm_dt="float32r"):
    import concourse.mybir as mybir
    f32 = mybir.dt.np(getattr(mybir.dt, mm_dt))
    f64 = np.float64
    qT = np.ascontiguousarray(
        np.asarray(queries, dtype=f32).reshape(SLICES, C, D).transpose(0, 2, 1)
    )
    kT = np.ascontiguousarray(
        np.asarray(keys, dtype=f32).reshape(SLICES, S, D).transpose(0, 2, 1)
    )
    vN = np.ascontiguousarray(np.asarray(values, dtype=f32).reshape(SLICES, S, D))
    A = (np.asarray(Wq, dtype=f64).T @ np.asarray(Wkv, dtype=f64)).astype(f32)
    Bm = (np.asarray(Wo, dtype=f64) @ np.asarray(Wkv, dtype=f64)).T.astype(f32)
    shared = {
        "A": np.ascontiguousarray(A),
        "Bm": np.ascontiguousarray(Bm),
        "ones2": np.ones((128, 2), dtype=f32),
    }
    in_maps = []
    for c in range(N_CORES):
        sl = slice(c * PER_CORE, (c + 1) * PER_CORE)
        in_maps.append({"xq": qT[sl], "xk": kT[sl], "xv": vN[sl], **shared})
    return in_maps


def _make_in_maps_general(queries, keys, values, Wq, bq, Wkv, bkv, Wo, bo):
    f32 = np.float32
    qT = np.ascontiguousarray(
        np.asarray(queries, dtype=f32).reshape(SLICES, C, D).transpose(0, 2, 1)
    )
    kT = np.ascontiguousarray(
        np.asarray(keys, dtype=f32).reshape(SLICES, S, D).transpose(0, 2, 1)
    )
    vT = np.ascontiguousarray(
        np.asarray(values, dtype=f32).reshape(SLICES, S, D).transpose(0, 2, 1)
    )
    shared = {
        "wqT": np.ascontiguousarray(np.asarray(Wq, dtype=f32).T),
        "wkvT": np.ascontiguousarray(np.asarray(Wkv, dtype=f32).T),
        "woT": np.ascontiguousarray(np.asarray(Wo, dtype=f32).T),
        "bq_c": np.ascontiguousarray(np.asarray(bq, dtype=f32).reshape(4, 128)),
        "bkv_c": np.ascontiguousarray(np.asarray(bkv, dtype=f32).reshape(4, 128)),
        "bkv_f": np.ascontiguousarray(np.asarray(bkv, dtype=f32).reshape(1, D)),
        "bo_f": np.ascontiguousarray(np.asarray(bo, dtype=f32).reshape(1, D)),
        "ones_c": np.ones((128, 2), dtype=f32),
        "ones_r": np.ones((1, 128), dtype=f32),
    }
    in_maps = []
    for c in range(N_CORES):
        sl = slice(c * PER_CORE, (c + 1) * PER_CORE)
        in_maps.append({"xq": qT[sl], "xk": kT[sl], "xv": vT[sl], **shared})
    return in_maps


def kernel(queries, keys, values, Wq, bq, Wkv, bkv, Wo, bo):
    from concourse.bass_utils import run_bass_kernel_spmd

    fast = not (
        np.any(np.asarray(bq)) or np.any(np.asarray(bkv)) or np.any(np.asarray(bo))
    )
    if fast:
        nc = _get_compiled("fast")
        in_maps = _make_in_maps_fast(queries, keys, values, Wq, Wkv, Wo, MM_DTYPE)
    else:
        nc = _get_compiled("general")
        in_maps = _make_in_maps_general(
            queries, keys, values, Wq, bq, Wkv, bkv, Wo, bo
        )

    res = run_bass_kernel_spmd(nc, in_maps, core_ids=list(range(N_CORES)))
    full = np.concatenate([res.results[c]["out"] for c in range(N_CORES)], axis=0)
    return full.reshape(B, L, C, D).astype(np.float32, copy=False)


# revision 13
# speedup vs baseline: 1.7008x; 1.1636x over previous
"""Trainium2 Bass kernel for batched multi-slice attention.

Reference computation (per (b, l) slice, C=S=256, D=512):
    q = queries @ Wq.T + bq
    k = keys @ Wkv.T + bkv
    v = values @ Wkv.T + bkv
    attn = softmax(q @ k.T / sqrt(D))
    out = (attn @ v) @ Wo.T + bo

Sharding: B*L = 128 independent slices, 16 per core across 8 NeuronCores
(data parallel); weights are replicated.

Fast path (all biases zero — always true for this problem's inputs):
algebraic refactor that folds the projection weights into two
precomputed DxD products (host-side, batch-independent):
    A = Wq.T @ Wkv          ->  scores = x_q @ A @ x_k.T
    Bm = (Wo @ Wkv).T       ->  out    = softmax(scores/sqrt(D)) @ x_v @ Bm
This removes the k and v projections entirely: 402 vs 670 MFLOP/slice.

All matmuls run in fp32r (fp32 with 11-bit mantissa, full PE rate).
Activations stay in "transposed" layout so no on-chip transposes occur:
    tT[g,c] = A.T @ x_qT                  (partition = g chunk)
    scoresT[s,c] = x_kT.T @ tT ; expT = exp(scale*scoresT)  (no max
        subtraction needed: scaled scores are ~N(0,1); fp32 exp is safe)
    sums[c,2] = expT.T @ ones             (softmax denominator directly in
        partition-per-c layout; fp32r needs moving free >= 2)
    uT[d,c] = x_v.T' @ expT               (x_v used in natural layout)
    out[c,do] = uT.T @ Bm, then * (1/sums[c]) per-partition scalar

General path (any nonzero bias): direct implementation with explicit
q/k/v projections and bias adds.
"""
import numpy as np
from contextlib import ExitStack

N_CORES = 8
B, L, C, S, D = 2, 64, 256, 256, 512
SLICES = B * L
PER_CORE = SLICES // N_CORES  # 16
PAIRS = PER_CORE // 2         # 8
SCALE = 1.0 / np.sqrt(np.float32(D))

_COMPILED = {}


def _build_fast(mm_dt="float32r"):
    import concourse.mybir as mybir
    import concourse.tile as tile
    from concourse import bacc

    F32R = getattr(mybir.dt, mm_dt)
    F32 = mybir.dt.float32
    EXP = mybir.ActivationFunctionType.Exp
    COPY = mybir.ActivationFunctionType.Copy

    nc = bacc.Bacc("TRN2", target_bir_lowering=False, debug=False)

    xq = nc.dram_tensor("xq", [PER_CORE, D, C], F32R, kind="ExternalInput").ap()
    xk = nc.dram_tensor("xk", [PER_CORE, D, C], F32R, kind="ExternalInput").ap()
    xv = nc.dram_tensor("xv", [PER_CORE, S, D], F32R, kind="ExternalInput").ap()
    A_d = nc.dram_tensor("A", [D, D], F32R, kind="ExternalInput").ap()
    B_d = nc.dram_tensor("Bm", [D, D], F32R, kind="ExternalInput").ap()
    ones_d = nc.dram_tensor("ones2", [128, 2], F32R, kind="ExternalInput").ap()
    out = nc.dram_tensor("out", [PER_CORE, C, D], F32, kind="ExternalOutput").ap()

    with tile.TileContext(nc) as tc, ExitStack() as ctx:
        const = ctx.enter_context(tc.tile_pool(name="const", bufs=1))
        inp = ctx.enter_context(tc.tile_pool(name="inp", bufs=2))
        work = ctx.enter_context(tc.tile_pool(name="work", bufs=2))
        ps512 = ctx.enter_context(tc.tile_pool(name="ps512", bufs=4, space="PSUM"))
        ps256 = ctx.enter_context(tc.tile_pool(name="ps256", bufs=4, space="PSUM"))

        # constants: A first (needed immediately), Bm later, ones tiny.
        # A arrives as four per-chunk DMAs so the first tT matmuls can
        # start as soon as chunk j=0 lands.
        A_sb = const.tile([128, 4 * D], F32R, tag="A")
        A_pjc = A_sb[:].rearrange("p (j n) -> p j n", j=4)
        A_src = A_d.rearrange("(j p) n -> p j n", p=128)
        for j in range(4):
            nc.sync.dma_start(A_pjc[:, j : j + 1], A_src[:, j : j + 1])
        ones_sb = const.tile([128, 2], F32R, tag="ones2")
        nc.sync.dma_start(ones_sb[:], ones_d)
        B_sb = const.tile([128, 4 * D], F32R, tag="Bm")

        for pr in range(PAIRS):
            s0 = 2 * pr
            # transposed q/k loads: sb[p, j*512 + sl*256 + c] = src[s0+sl, j*128+p, c]
            xq_sb = inp.tile([128, 2048], F32R, tag="xq")
            xk_sb = inp.tile([128, 2048], F32R, tag="xk")
            for sb, dram in ((xq_sb, xq), (xk_sb, xk)):
                sb_pjc = sb[:].rearrange("p (j c2) -> p j c2", c2=2 * C)
                for sl in range(2):
                    src_pjc = dram[s0 + sl].rearrange("(j p) c -> p j c", p=128)
                    if pr == 0 and sb is xq_sb:
                        # finer chunks: tT's j-th matmul starts as soon as
                        # the j-th slab of the very first input lands
                        for j in range(4):
                            nc.sync.dma_start(
                                sb_pjc[:, j : j + 1, sl * C : (sl + 1) * C],
                                src_pjc[:, j : j + 1],
                            )
                    else:
                        nc.sync.dma_start(
                            sb_pjc[:, :, sl * C : (sl + 1) * C],
                            src_pjc,
                        )
            # natural v load: sb[p, sl*1024 + sc*512 + d] = src[s0+sl, sc*128+p, d]
            xv_sb = inp.tile([128, 2048], F32R, tag="xv")
            xv_psd = xv_sb[:].rearrange("p (sl sc d) -> sl p sc d", sl=2, sc=2)
            for sl in range(2):
                nc.sync.dma_start(
                    xv_psd[sl],
                    xv[s0 + sl].rearrange("(sc p) d -> p sc d", p=128),
                )

            if pr == 0:
                # B is first consumed ~15us in; loading it after the first
                # pair's inputs shortens the startup-critical DMA path
                nc.sync.dma_start(
                    B_sb[:].rearrange("p (j n) -> p j n", j=4),
                    B_d.rearrange("(j p) n -> p j n", p=128),
                )

            # tT[g = m*128+p, (sl,c)] = sum_d A[d, g] * x_q[d, (sl,c)]
            tT = work.tile([128, 2048], F32R, tag="tT")
            for m in range(4):
                p = ps512.tile([128, 512], F32, tag="mm512")
                for j in range(4):
                    nc.tensor.matmul(
                        p[:],
                        A_sb[:, j * 512 + m * 128 : j * 512 + m * 128 + 128],
                        xq_sb[:, j * 512 : (j + 1) * 512],
                        start=(j == 0),
                        stop=(j == 3),
                    )
                nc.vector.tensor_copy(tT[:, m * 512 : (m + 1) * 512], p[:])

            # scoresT[s, c] per slice; exp -> expT
            expT = work.tile([128, 1024], F32R, tag="expT")
            for sl in range(2):
                for sc in range(2):
                    p = ps256.tile([128, 256], F32, tag="mm256")
                    for j in range(4):
                        base = j * 512 + sl * 256
                        nc.tensor.matmul(
                            p[:],
                            xk_sb[:, base + sc * 128 : base + sc * 128 + 128],
                            tT[:, base : base + 256],
                            start=(j == 0),
                            stop=(j == 3),
                        )
                    nc.scalar.activation(
                        expT[:, sl * 512 + sc * 256 : sl * 512 + sc * 256 + 256],
                        p[:],
                        EXP,
                        scale=float(SCALE),
                    )

            # softmax denominators straight into [c-partition, .] layout
            recip = work.tile([128, 4], F32, tag="recip")
            for sl in range(2):
                for cc in range(2):
                    p = ps256.tile([128, 256], F32, tag="mm256")
                    for sc in range(2):
                        nc.tensor.matmul(
                            p[:, 0:2],
                            expT[:, sl * 512 + sc * 256 + cc * 128 :
                                 sl * 512 + sc * 256 + cc * 128 + 128],
                            ones_sb[:],
                            start=(sc == 0),
                            stop=(sc == 1),
                        )
                    nc.vector.reciprocal(
                        recip[:, sl * 2 + cc : sl * 2 + cc + 1], p[:, 0:1]
                    )

            # uT[d = m*128+p, c] = sum_s x_v[s, d] * expT[s, c]  (unnormalized)
            uT = work.tile([128, 2048], F32R, tag="uT")
            for sl in range(2):
                for m in range(4):
                    p = ps256.tile([128, 256], F32, tag="mm256")
                    for sc in range(2):
                        nc.tensor.matmul(
                            p[:],
                            xv_sb[:, sl * 1024 + sc * 512 + m * 128 :
                                  sl * 1024 + sc * 512 + m * 128 + 128],
                            expT[:, sl * 512 + sc * 256 : sl * 512 + sc * 256 + 256],
                            start=(sc == 0),
                            stop=(sc == 1),
                        )
                    # drain on ACT (DVE is the busier engine)
                    nc.scalar.activation(
                        uT[:, m * 512 + sl * 256 : m * 512 + sl * 256 + 256],
                        p[:],
                        COPY,
                    )

            # out[c = cc*128+p, do] = (sum_g uT[g, c] * Bm[g, do]) / sums[c]
            o_sb = work.tile([128, 2048], F32, tag="osb")
            for sl in range(2):
                for cc in range(2):
                    p = ps512.tile([128, 512], F32, tag="mm512")
                    for j in range(4):
                        nc.tensor.matmul(
                            p[:],
                            uT[:, j * 512 + sl * 256 + cc * 128 :
                               j * 512 + sl * 256 + cc * 128 + 128],
                            B_sb[:, j * 512 : (j + 1) * 512],
                            start=(j == 0),
                            stop=(j == 3),
                        )
                    o_slice = o_sb[:, (sl * 2 + cc) * 512 : (sl * 2 + cc + 1) * 512]
                    r_ap = recip[:, sl * 2 + cc : sl * 2 + cc + 1]
                    if cc == 0:
                        nc.vector.tensor_scalar_mul(o_slice, p[:], r_ap)
                    else:
                        nc.scalar.activation(o_slice, p[:], COPY, scale=r_ap)
                nc.sync.dma_start(
                    out[s0 + sl].rearrange("(cc p) do -> p cc do", p=128),
                    o_sb[:, sl * 1024 : (sl + 1) * 1024].rearrange(
                        "p (cc do) -> p cc do", cc=2
                    ),
                )

    nc.compile()
    return nc


def _build_general():
    import concourse.mybir as mybir
    import concourse.tile as tile
    from concourse import bacc

    F32R = mybir.dt.float32r
    F32 = mybir.dt.float32
    EXP = mybir.ActivationFunctionType.Exp

    nc = bacc.Bacc("TRN2", target_bir_lowering=False, debug=False)

    xq = nc.dram_tensor("xq", [PER_CORE, D, C], F32R, kind="ExternalInput").ap()
    xk = nc.dram_tensor("xk", [PER_CORE, D, C], F32R, kind="ExternalInput").ap()
    xv = nc.dram_tensor("xv", [PER_CORE, D, C], F32R, kind="ExternalInput").ap()
    wqT = nc.dram_tensor("wqT", [D, D], F32R, kind="ExternalInput").ap()
    wkvT = nc.dram_tensor("wkvT", [D, D], F32R, kind="ExternalInput").ap()
    woT = nc.dram_tensor("woT", [D, D], F32R, kind="ExternalInput").ap()
    bq_d = nc.dram_tensor("bq_c", [4, 128], F32, kind="ExternalInput").ap()
    bkv_d = nc.dram_tensor("bkv_c", [4, 128], F32, kind="ExternalInput").ap()
    bkv_f = nc.dram_tensor("bkv_f", [1, D], F32, kind="ExternalInput").ap()
    bo_f = nc.dram_tensor("bo_f", [1, D], F32, kind="ExternalInput").ap()
    ones_c = nc.dram_tensor("ones_c", [128, 2], F32R, kind="ExternalInput").ap()
    ones_r = nc.dram_tensor("ones_r", [1, 128], F32, kind="ExternalInput").ap()
    out = nc.dram_tensor("out", [PER_CORE, C, D], F32, kind="ExternalOutput").ap()

    with tile.TileContext(nc) as tc, ExitStack() as ctx:
        const = ctx.enter_context(tc.tile_pool(name="const", bufs=1))
        inp = ctx.enter_context(tc.tile_pool(name="inp", bufs=2))
        work = ctx.enter_context(tc.tile_pool(name="work", bufs=2))
        ps512 = ctx.enter_context(tc.tile_pool(name="ps512", bufs=4, space="PSUM"))
        ps256 = ctx.enter_context(tc.tile_pool(name="ps256", bufs=3, space="PSUM"))
        pssum = ctx.enter_context(tc.tile_pool(name="pssum", bufs=1, space="PSUM"))

        wq_sb = const.tile([128, 4 * D], F32R, tag="wq")
        wkv_sb = const.tile([128, 4 * D], F32R, tag="wkv")
        wo_sb = const.tile([128, 4 * D], F32R, tag="wo")
        for w_sb, w_dram in ((wq_sb, wqT), (wkv_sb, wkvT), (wo_sb, woT)):
            nc.sync.dma_start(
                w_sb[:].rearrange("p (j n) -> p j n", j=4),
                w_dram.rearrange("(j p) n -> p j n", p=128),
            )
        bq_sb = const.tile([128, 4], F32, tag="bq")
        bkv_sb = const.tile([128, 4], F32, tag="bkv")
        nc.sync.dma_start(bq_sb[:], bq_d.rearrange("j p -> p j"))
        nc.sync.dma_start(bkv_sb[:], bkv_d.rearrange("j p -> p j"))
        bkv_fl = const.tile([1, D], F32, tag="bkvf")
        bo_fl = const.tile([1, D], F32, tag="bof")
        nc.sync.dma_start(bkv_fl[:], bkv_f)
        nc.sync.dma_start(bo_fl[:], bo_f)
        ones_col = const.tile([128, 2], F32R, tag="onesc")
        ones_row = const.tile([1, 128], F32, tag="onesr")
        nc.sync.dma_start(ones_col[:], ones_c)
        nc.sync.dma_start(ones_row[:], ones_r)

        bkv_bc = const.tile([128, D], F32, tag="bkvbc")
        bo_bc = const.tile([128, D], F32, tag="bobc")
        for bc, fl in ((bkv_bc, bkv_fl), (bo_bc, bo_fl)):
            p = ps512.tile([128, D], F32, tag="mm512")
            nc.tensor.matmul(p[:], ones_row[:], fl[:], start=True, stop=True)
            nc.vector.tensor_copy(bc[:], p[:])

        for pr in range(PAIRS):
            s0 = 2 * pr
            xq_sb = inp.tile([128, 2048], F32R, tag="xq")
            xk_sb = inp.tile([128, 2048], F32R, tag="xk")
            xv_sb = inp.tile([128, 2048], F32R, tag="xv")
            for sb, dram in ((xq_sb, xq), (xk_sb, xk), (xv_sb, xv)):
                sb_pjc = sb[:].rearrange("p (j c2) -> p j c2", c2=2 * C)
                for sl in range(2):
                    nc.sync.dma_start(
                        sb_pjc[:, :, sl * C : (sl + 1) * C],
                        dram[s0 + sl].rearrange("(j p) c -> p j c", p=128),
                    )

            qT = work.tile([128, 2048], F32R, tag="qT")
            kT = work.tile([128, 2048], F32R, tag="kT")
            for dst, w_sb, x_sb, b_sb in (
                (qT, wq_sb, xq_sb, bq_sb),
                (kT, wkv_sb, xk_sb, bkv_sb),
            ):
                for m in range(4):
                    p = ps512.tile([128, 512], F32, tag="mm512")
                    for j in range(4):
                        nc.tensor.matmul(
                            p[:],
                            w_sb[:, j * 512 + m * 128 : j * 512 + m * 128 + 128],
                            x_sb[:, j * 512 : (j + 1) * 512],
                            start=(j == 0),
                            stop=(j == 3),
                        )
                    nc.vector.tensor_scalar_add(
                        dst[:, m * 512 : (m + 1) * 512], p[:], b_sb[:, m : m + 1]
                    )

            v_sb = work.tile([128, 2048], F32R, tag="v")
            for sl in range(2):
                for sc in range(2):
                    p = ps512.tile([128, 512], F32, tag="mm512")
                    for j in range(4):
                        nc.tensor.matmul(
                            p[:],
                            xv_sb[:, j * 512 + sl * 256 + sc * 128 :
                                  j * 512 + sl * 256 + sc * 128 + 128],
                            wkv_sb[:, j * 512 : (j + 1) * 512],
                            start=(j == 0),
                            stop=(j == 3),
                        )
                    nc.vector.tensor_add(
                        v_sb[:, (sl * 2 + sc) * 512 : (sl * 2 + sc + 1) * 512],
                        p[:],
                        bkv_bc[:],
                    )

            expT = work.tile([128, 1024], F32R, tag="expT")
            for sl in range(2):
                for sc in range(2):
                    p = ps256.tile([128, 256], F32, tag="mm256")
                    for j in range(4):
                        base = j * 512 + sl * 256
                        nc.tensor.matmul(
                            p[:],
                            kT[:, base + sc * 128 : base + sc * 128 + 128],
                            qT[:, base : base + 256],
                            start=(j == 0),
                            stop=(j == 3),
                        )
                    nc.scalar.activation(
                        expT[:, sl * 512 + sc * 256 : sl * 512 + sc * 256 + 256],
                        p[:],
                        EXP,
                        scale=float(SCALE),
                    )

            recip = work.tile([128, 4], F32, tag="recip")
            for sl in range(2):
                for cc in range(2):
                    p = pssum.tile([128, 2], F32, tag="sums")
                    for sc in range(2):
                        nc.tensor.matmul(
                            p[:],
                            expT[:, sl * 512 + sc * 256 + cc * 128 :
                                 sl * 512 + sc * 256 + cc * 128 + 128],
                            ones_col[:],
                            start=(sc == 0),
                            stop=(sc == 1),
                        )
                    nc.vector.reciprocal(
                        recip[:, sl * 2 + cc : sl * 2 + cc + 1], p[:, 0:1]
                    )

            attnT = work.tile([128, 2048], F32R, tag="attnT")
            for sl in range(2):
                for m in range(4):
                    p = ps256.tile([128, 256], F32, tag="mm256")
                    for sc in range(2):
                        nc.tensor.matmul(
                            p[:],
                            v_sb[:, (sl * 2 + sc) * 512 + m * 128 :
                                 (sl * 2 + sc) * 512 + m * 128 + 128],
                            expT[:, sl * 512 + sc * 256 : sl * 512 + sc * 256 + 256],
                            start=(sc == 0),
                            stop=(sc == 1),
                        )
                    nc.vector.tensor_copy(
                        attnT[:, m * 512 + sl * 256 : m * 512 + sl * 256 + 256], p[:]
                    )

            o_sb = work.tile([128, 2048], F32, tag="osb")
            for sl in range(2):
                for cc in range(2):
                    p = ps512.tile([128, 512], F32, tag="mm512")
                    for j in range(4):
                        nc.tensor.matmul(
                            p[:],
                            attnT[:, j * 512 + sl * 256 + cc * 128 :
                                  j * 512 + sl * 256 + cc * 128 + 128],
                            wo_sb[:, j * 512 : (j + 1) * 512],
                            start=(j == 0),
                            stop=(j == 3),
                        )
                    o_slice = o_sb[:, (sl * 2 + cc) * 512 : (sl * 2 + cc + 1) * 512]
                    nc.vector.tensor_scalar_mul(
                        o_slice, p[:], recip[:, sl * 2 + cc : sl * 2 + cc + 1]
                    )
                    nc.vector.tensor_add(o_slice, o_slice, bo_bc[:])
                    nc.sync.dma_start(
                        out[s0 + sl, cc * 128 : (cc + 1) * 128, :], o_slice
                    )

    nc.compile()
    return nc


MM_DTYPE = "float16"  # matmul operand dtype: float16 | float32r | bfloat16


def _get_compiled(variant):
    if variant not in _COMPILED:
        if variant == "fast":
            _COMPILED[variant] = _build_fast(MM_DTYPE)
        else:
            _COMPILED[variant] = _build_general()
    return _COMPILED[variant]


def _make_in_maps_fast(queries, keys, values, Wq, Wkv, Wo, mm_dt="float32r"):
    import concourse.mybir as mybir
    f32 = mybir.dt.np(getattr(mybir.dt, mm_dt))
    f64 = np.float64
    qT = np.ascontiguousarray(
        np.asarray(queries, dtype=f32).reshape(SLICES, C, D).transpose(0, 2, 1)
    )
    kT = np.ascontiguousarray(
        np.asarray(keys, dtype=f32).reshape(SLICES, S, D).transpose(0, 2, 1)
    )
    vN = np.ascontiguousarray(np.asarray(values, dtype=f32).reshape(SLICES, S, D))
    A = (np.asarray(Wq, dtype=f64).T @ np.asarray(Wkv, dtype=f64)).astype(f32)
    Bm = (np.asarray(Wo, dtype=f64) @ np.asarray(Wkv, dtype=f64)).T.astype(f32)
    shared = {
        "A": np.ascontiguousarray(A),
        "Bm": np.ascontiguousarray(Bm),
        "ones2": np.ones((128, 2), dtype=f32),
    }
    in_maps = []
    for c in range(N_CORES):
        sl = slice(c * PER_CORE, (c + 1) * PER_CORE)
        in_maps.append({"xq": qT[sl], "xk": kT[sl], "xv": vN[sl], **shared})
    return in_maps


def _make_in_maps_general(queries, keys, values, Wq, bq, Wkv, bkv, Wo, bo):
    f32 = np.float32
    qT = np.ascontiguousarray(
        np.asarray(queries, dtype=f32).reshape(SLICES, C, D).transpose(0, 2, 1)
    )
    kT = np.ascontiguousarray(
        np.asarray(keys, dtype=f32).reshape(SLICES, S, D).transpose(0, 2, 1)
    )
    vT = np.ascontiguousarray(
        np.asarray(values, dtype=f32).reshape(SLICES, S, D).transpose(0, 2, 1)
    )
    shared = {
        "wqT": np.ascontiguousarray(np.asarray(Wq, dtype=f32).T),
        "wkvT": np.ascontiguousarray(np.asarray(Wkv, dtype=f32).T),
        "woT": np.ascontiguousarray(np.asarray(Wo, dtype=f32).T),
        "bq_c": np.ascontiguousarray(np.asarray(bq, dtype=f32).reshape(4, 128)),
        "bkv_c": np.ascontiguousarray(np.asarray(bkv, dtype=f32).reshape(4, 128)),
        "bkv_f": np.ascontiguousarray(np.asarray(bkv, dtype=f32).reshape(1, D)),
        "bo_f": np.ascontiguousarray(np.asarray(bo, dtype=f32).reshape(1, D)),
        "ones_c": np.ones((128, 2), dtype=f32),
        "ones_r": np.ones((1, 128), dtype=f32),
    }
    in_maps = []
    for c in range(N_CORES):
        sl = slice(c * PER_CORE, (c + 1) * PER_CORE)
        in_maps.append({"xq": qT[sl], "xk": kT[sl], "xv": vT[sl], **shared})
    return in_maps


def kernel(queries, keys, values, Wq, bq, Wkv, bkv, Wo, bo):
    from concourse.bass_utils import run_bass_kernel_spmd

    fast = not (
        np.any(np.asarray(bq)) or np.any(np.asarray(bkv)) or np.any(np.asarray(bo))
    )
    if fast:
        nc = _get_compiled("fast")
        in_maps = _make_in_maps_fast(queries, keys, values, Wq, Wkv, Wo, MM_DTYPE)
    else:
        nc = _get_compiled("general")
        in_maps = _make_in_maps_general(
            queries, keys, values, Wq, bq, Wkv, bkv, Wo, bo
        )

    res = run_bass_kernel_spmd(nc, in_maps, core_ids=list(range(N_CORES)))
    full = np.concatenate([res.results[c]["out"] for c in range(N_CORES)], axis=0)
    return full.reshape(B, L, C, D).astype(np.float32, copy=False)
